# revision 23
# baseline (speedup 1.0000x reference)
"""Trainium2 Bass kernel for nn_BulkSpaceGenerator.

Math: the fast-marching scan g_k = g_{k-1} + (1/(k+1))(c_k - g_{k-1}) starting
from c_0 yields the running mean g_k = mean(c_0..c_k); the mean over k of those
is sum_j w_j c_j with w_j = (1/K)(H_K - H_j) (harmonic numbers). Since
c_j = tokens @ W[:, j*D:(j+1)*D] + b[j*D:(j+1)*D], the whole module is

    out = tokens @ W_eff + b_eff,   W_eff = sum_j w_j W_j,  b_eff = sum_j w_j b_j

The kernel folds W -> W_eff on-device (DVE) and runs the (8192x1024)@(1024x1024)
matmul on the PE array, sharded over 8 cores as 4 feature-shards x 2
token-shards (minimizes per-core HBM traffic: W_slice + tokens/2 + out/8).

Layout per core (f in 0..3, t in 0..1, core = f*2 + t):
  tokT : (1024, 4096) f32  -- tokens^T slice, columns t*4096:(t+1)*4096
  wsl  : (1024, 2560) f32  -- W[:, j*1024 + f*256 : j*1024 + (f+1)*256], j-major
  bsl  : (256, 10)    f32  -- b[j*1024 + f*256 + d] transposed to (d, j)
  outT : (256, 4096)  f32  -- out^T slice (host reassembles full (4,2048,1024))
"""

import os
from contextlib import ExitStack

import numpy as np

import concourse.bass as bass
import concourse.tile as tile
from concourse import bacc, mybir
from concourse.bass_utils import run_bass_kernel_spmd

D_MODEL = 1024
BULK_DIM = 10
B, N = 4, 2048
BN = B * N                     # 8192 tokens
NCORES = 8
F_SHARDS = 4                   # feature shards (d dimension)
T_SHARDS = 2                   # token shards
DS = D_MODEL // F_SHARDS       # 256 output features per core
MS = BN // T_SHARDS            # 4096 tokens per core
KT = D_MODEL // 128            # 8 contraction k-tiles
DT = DS // 128                 # 2 output d-tiles of 128 per core
MCHUNK = 512                   # moving free dim per matmul
NMI = MS // MCHUNK             # 8 m-chunks per core

# w_j = (1/K) * (H_K - H_j), H_j = sum_{i=1..j} 1/i
_H = np.cumsum(1.0 / np.arange(1, BULK_DIM + 1))
W_COEF = ((_H[-1] - np.concatenate([[0.0], _H[:-1]])) / BULK_DIM).tolist()

# mode: "f32r" | "f32" | "bf16" keep f32 inputs on the wire; "f16" ships
# tokens and W as fp16 (half the load bytes, ~3.6e-4 rel err vs 1.5e-4 f32r)
# "cc": 8-way token shard + cooperative W_eff fold shared via AllGather
#       (dead: collectives have ~85us fixed cost under axon NRT)
# "v2": f16 wire + f16 out, kt-outer streaming in two m-phases
MODE = os.environ.get("BULK_KERNEL_MODE", "v2")

_BUILD_CACHE = {}

# ---------------- v2: f16 wire, kt-outer streaming, f16 out ----------------
# Same 4 feature-shards x 2 token-shards as f16 mode, but:
#  - DMA order per kt: W rows-chunk then phase-A token chunk, so the PE
#    starts accumulating at ~3us and chases the incoming stream
#  - W_eff fold runs in f16 (2x DVE rate); accuracy cost ~1e-4, fine
#  - psum is 8 banks of (128, 512): phase A = m 0..2047 kt-outer,
#    phase B = m 2048..4095 kt-outer once A's banks evict
#  - output ships f16 (halves store traffic; host upcasts)


def _build_v2() -> bass.Bass:
    f16 = mybir.dt.float16
    f32 = mybir.dt.float32

    nc = bacc.Bacc("TRN2", target_bir_lowering=False, debug=False,
                   num_devices=NCORES)
    tokT = nc.dram_tensor("tokT", [D_MODEL, MS], f16,
                          kind="ExternalInput").ap()
    wsl = nc.dram_tensor("wsl", [D_MODEL, BULK_DIM * DS], f16,
                         kind="ExternalInput").ap()
    bsl = nc.dram_tensor("bsl", [DS, BULK_DIM], f32, kind="ExternalInput").ap()
    outT = nc.dram_tensor("outT", [DS, MS], f16, kind="ExternalOutput").ap()

    HM = MS // 2                # 2048 tokens per phase
    NPH_MI = HM // MCHUNK       # 4 m-chunks per phase

    with tile.TileContext(nc) as tc, ExitStack() as ctx:
        sb = ctx.enter_context(tc.tile_pool(name="sb", bufs=1))
        out_pool = ctx.enter_context(tc.tile_pool(name="osb", bufs=4))
        psum_pool = ctx.enter_context(
            tc.tile_pool(name="psum", bufs=8, space="PSUM"))

        mult = mybir.AluOpType.mult
        add = mybir.AluOpType.add

        # PE warm operands
        zf = sb.tile([128, 512], f32)
        nc.vector.memset(zf[:], 0.0)
        zmm = sb.tile([128, 128], f16)
        nc.scalar.copy(zmm[:], zf[:, 0:128])
        zrhs = sb.tile([128, 512], f16)
        nc.scalar.copy(zrhs[:], zf[:])

        # ---- input stream (sync queue, program order == priority) ----
        wrs, toks = [], []
        for kt in range(KT):
            ksl = slice(kt * 128, (kt + 1) * 128)
            wr = sb.tile([128, BULK_DIM * DS], f16, name=f"wr{kt}")
            nc.sync.dma_start(wr[:], wsl[ksl, :])
            wrs.append(wr)
            tk = sb.tile([128, MS], f16, name=f"tk{kt}")
            nc.sync.dma_start(tk[:, 0:HM], tokT[ksl, 0:HM])
            toks.append(tk)
        for kt in range(KT):
            ksl = slice(kt * 128, (kt + 1) * 128)
            nc.sync.dma_start(toks[kt][:, HM:], tokT[ksl, HM:])

        # ---- fold W_eff per kt on DVE, all-f16 (chases the W DMAs) ----
        weffs = []
        for kt in range(KT):
            we = sb.tile([128, DS], f16, name=f"we{kt}")
            nc.vector.tensor_scalar_mul(we[:], wrs[kt][:, 0:DS], W_COEF[0])
            for j in range(1, BULK_DIM):
                nc.vector.scalar_tensor_tensor(
                    we[:], wrs[kt][:, j * DS:(j + 1) * DS], W_COEF[j],
                    we[:], mult, add)
            weffs.append(we)

        # ---- bias fold (f32, tiny) ----
        biases = []
        for dt_i in range(DT):
            bt = sb.tile([128, BULK_DIM], f32, name=f"bt{dt_i}")
            nc.scalar.dma_start(bt[:], bsl[dt_i * 128:(dt_i + 1) * 128, :])
            be = sb.tile([128, 1], f32, name=f"be{dt_i}")
            nc.vector.tensor_scalar_mul(be[:], bt[:, 0:1], W_COEF[0])
            for j in range(1, BULK_DIM):
                nc.vector.scalar_tensor_tensor(
                    be[:], bt[:, j:j + 1], W_COEF[j], be[:], mult, add)
            biases.append(be)

        def evict(ps, dt_i, msl):
            ot = out_pool.tile([128, MCHUNK], f16, name="ot", tag="ot")
            if dt_i == 0:
                nc.scalar.add(ot[:], ps[:], biases[dt_i][:])
            else:
                nc.vector.tensor_scalar_add(ot[:], ps[:], biases[dt_i][:, 0:1])
            nc.gpsimd.dma_start(outT[dt_i * 128:(dt_i + 1) * 128, msl], ot[:])

        # keep the PE HAM clock warm until the first weff lands
        ps_warm = psum_pool.tile([128, MCHUNK], f32, name="ps", tag="ps")
        for _ in range(16):
            nc.tensor.matmul(ps_warm[:], lhsT=zmm[:], rhs=zrhs[:],
                             start=False, stop=False)

        # ---- phase A: m 0..HM, kt-outer over 8 live psum groups ----
        psA = [ps_warm] + [
            psum_pool.tile([128, MCHUNK], f32, name="ps", tag="ps")
            for _ in range(NPH_MI * DT - 1)]
        for kt in range(KT):
            for dt_i in range(DT):
                for mi in range(NPH_MI):
                    nc.tensor.matmul(
                        psA[mi * DT + dt_i][:],
                        lhsT=weffs[kt][:, dt_i * 128:(dt_i + 1) * 128],
                        rhs=toks[kt][:, mi * MCHUNK:(mi + 1) * MCHUNK],
                        start=(kt == 0), stop=(kt == KT - 1))
        for mi in range(NPH_MI):
            for dt_i in range(DT):
                evict(psA[mi * DT + dt_i], dt_i,
                      slice(mi * MCHUNK, (mi + 1) * MCHUNK))

        # ---- phase B: m HM..MS ----
        # phase B: tokens are resident by now, so run single-group
        # accumulation chains and evict each group immediately -- spreads
        # the evict+store tail across phase B instead of piling it at the
        # end of the kernel.
        for mi in range(NPH_MI):
            for dt_i in range(DT):
                ps = psum_pool.tile([128, MCHUNK], f32, name="ps", tag="ps")
                m0 = HM + mi * MCHUNK
                for kt in range(KT):
                    nc.tensor.matmul(
                        ps[:],
                        lhsT=weffs[kt][:, dt_i * 128:(dt_i + 1) * 128],
                        rhs=toks[kt][:, m0:m0 + MCHUNK],
                        start=(kt == 0), stop=(kt == KT - 1))
                evict(ps, dt_i, slice(m0, m0 + MCHUNK))

    nc.compile()
    return nc


def _make_in_maps_v2(boundary_tokens, W_b2b, b_b2b, wsl3d=False):
    tok = np.ascontiguousarray(
        np.asarray(boundary_tokens, dtype=np.float32)
        .reshape(BN, D_MODEL).T.astype(np.float16))
    W = np.asarray(W_b2b, np.float32).astype(np.float16).reshape(
        D_MODEL, BULK_DIM, D_MODEL)
    b = np.asarray(b_b2b, np.float32).reshape(BULK_DIM, D_MODEL)
    wshape = ((D_MODEL, BULK_DIM, DS) if wsl3d
              else (D_MODEL, BULK_DIM * DS))
    in_maps = []
    for c in range(NCORES):
        f, t = divmod(c, T_SHARDS)
        dsl = slice(f * DS, (f + 1) * DS)
        in_maps.append({
            "tokT": np.ascontiguousarray(tok[:, t * MS:(t + 1) * MS]),
            "wsl": np.ascontiguousarray(
                W[:, :, dsl].reshape(*wshape)),
            "bsl": np.ascontiguousarray(b[:, dsl].T),
        })
    return in_maps


def _assemble_v2(results):
    out = np.empty((BN, D_MODEL), dtype=np.float32)
    for c in range(NCORES):
        f, t = divmod(c, T_SHARDS)
        out[t * MS:(t + 1) * MS, f * DS:(f + 1) * DS] = results[c]["outT"].T
    return out.reshape(B, N, D_MODEL)


# ------- v4: v2 + fold restructured to fit the DMA arrival cadence -------
# The W_eff fold is 2 ALU ops/elem; v2's 10-step STT chain costs 4.25us
# per kt-tile on the DVE vs the 3.26us DMA arrival cadence, so it paced
# the whole kernel. v4 folds each kt as:
#   vector : tmp[0:6] = W[0:6] * C  (one big tensor_mul vs a memset
#            coefficient tile), then an in-place pairwise add tree
#   scalar : tmp[6..9] = cj*Wj      (4 ACT scale-copies)
# ~2.8us/kt on vector, ~1.6us/kt on scalar (gpsimd can't run DVE ops on
# real TRN2 - ISA check rejects Pool-engine TensorScalar).


def _build_v3() -> bass.Bass:
    f16 = mybir.dt.float16
    f32 = mybir.dt.float32

    nc = bacc.Bacc("TRN2", target_bir_lowering=False, debug=False,
                   num_devices=NCORES)
    tokT = nc.dram_tensor("tokT", [D_MODEL, MS], f16,
                          kind="ExternalInput").ap()
    wsl = nc.dram_tensor("wsl", [D_MODEL, BULK_DIM, DS], f16,
                         kind="ExternalInput").ap()
    bsl = nc.dram_tensor("bsl", [DS, BULK_DIM], f32, kind="ExternalInput").ap()
    outT = nc.dram_tensor("outT", [DS, MS], f16, kind="ExternalOutput").ap()

    HM = MS // 2
    NPH_MI = HM // MCHUNK

    with tile.TileContext(nc) as tc, ExitStack() as ctx:
        sb = ctx.enter_context(tc.tile_pool(name="sb", bufs=1))
        tmp_pool = ctx.enter_context(tc.tile_pool(name="tmp", bufs=2))
        out_pool = ctx.enter_context(tc.tile_pool(name="osb", bufs=4))
        psum_pool = ctx.enter_context(
            tc.tile_pool(name="psum", bufs=8, space="PSUM"))

        mult = mybir.AluOpType.mult
        add = mybir.AluOpType.add

        zf = sb.tile([128, 512], f32)
        nc.vector.memset(zf[:], 0.0)
        zmm = sb.tile([128, 128], f16)
        nc.scalar.copy(zmm[:], zf[:, 0:128])
        zrhs = sb.tile([128, 512], f16)
        nc.scalar.copy(zrhs[:], zf[:])

        # coefficient tile for the one-op fold multiply (j = 0..5)
        cco = sb.tile([128, 6, DS], f16)
        for j in range(6):
            nc.vector.memset(cco[:, j, :], float(W_COEF[j]))

        # W chunks run one step ahead of the phase-A token chunks so the
        # fold is never the late dependency of a kt batch.
        wrs = [sb.tile([128, BULK_DIM, DS], f16, name=f"wr{kt}")
               for kt in range(KT)]
        toks = [sb.tile([128, MS], f16, name=f"tk{kt}") for kt in range(KT)]

        def ksl(kt):
            return slice(kt * 128, (kt + 1) * 128)

        nc.sync.dma_start(wrs[0][:], wsl[ksl(0), :, :])
        for kt in range(1, KT):
            nc.sync.dma_start(wrs[kt][:], wsl[ksl(kt), :, :])
            nc.sync.dma_start(toks[kt - 1][:, 0:HM], tokT[ksl(kt - 1), 0:HM])
        nc.sync.dma_start(toks[KT - 1][:, 0:HM], tokT[ksl(KT - 1), 0:HM])

        # ---- fold: big-op multiply + pairwise add tree ----
        weffs = []
        gate = None
        for kt in range(KT):
            wr = wrs[kt]
            tmp = tmp_pool.tile([128, BULK_DIM, DS], f16, name="tmp",
                                tag="tmp")
            nc.vector.tensor_mul(tmp[:, 0:6, :], wr[:, 0:6, :], cco[:])
            for j in range(6, BULK_DIM):
                act = nc.scalar.mul(tmp[:, j, :], wr[:, j, :], W_COEF[j])
                if kt == KT - 1 and j == BULK_DIM - 1:
                    gate = act
            nc.vector.tensor_add(tmp[:, 0:5, :], tmp[:, 0:5, :],
                                 tmp[:, 5:10, :])
            nc.vector.tensor_add(tmp[:, 0:2, :], tmp[:, 0:2, :],
                                 tmp[:, 2:4, :])
            nc.vector.tensor_add(tmp[:, 0, :], tmp[:, 0, :], tmp[:, 1, :])
            we = sb.tile([128, DS], f16, name=f"we{kt}")
            nc.vector.tensor_add(we[:], tmp[:, 0, :], tmp[:, 4, :])
            weffs.append(we)

        # phase-B tokens held back until the phase-A stream drains (the
        # kt=7 fold ACT fires right as the last A bytes land); keeps the
        # A stream undiluted -- the DMA queue round-robins bandwidth
        # across all outstanding transfers.
        from concourse.tile_rust import add_dep_helper as _adh
        for kt in range(KT):
            db = nc.sync.dma_start(toks[kt][:, HM:], tokT[ksl(kt), HM:])
            _adh(db.ins, gate.ins, sync=True)

        biases = []
        for dt_i in range(DT):
            bt = sb.tile([128, BULK_DIM], f32, name=f"bt{dt_i}")
            nc.scalar.dma_start(bt[:], bsl[dt_i * 128:(dt_i + 1) * 128, :])
            be = sb.tile([128, 1], f32, name=f"be{dt_i}")
            nc.vector.tensor_scalar_mul(be[:], bt[:, 0:1], W_COEF[0])
            for j in range(1, BULK_DIM):
                nc.vector.scalar_tensor_tensor(
                    be[:], bt[:, j:j + 1], W_COEF[j], be[:], mult, add)
            biases.append(be)

        def evict(ps, dt_i, msl):
            ot = out_pool.tile([128, MCHUNK], f16, name="ot", tag="ot")
            if dt_i == 0:
                nc.scalar.add(ot[:], ps[:], biases[dt_i][:])
            else:
                nc.vector.tensor_scalar_add(ot[:], ps[:], biases[dt_i][:, 0:1])
            nc.sync.dma_start(outT[dt_i * 128:(dt_i + 1) * 128, msl], ot[:])

        ps_warm = psum_pool.tile([128, MCHUNK], f32, name="ps", tag="ps")
        for _ in range(24):
            nc.tensor.matmul(ps_warm[:], lhsT=zmm[:], rhs=zrhs[:],
                             start=False, stop=False)

        psA = [ps_warm] + [
            psum_pool.tile([128, MCHUNK], f32, name="ps", tag="ps")
            for _ in range(NPH_MI * DT - 1)]
        for kt in range(KT):
            for dt_i in range(DT):
                for mi in range(NPH_MI):
                    nc.tensor.matmul(
                        psA[mi * DT + dt_i][:],
                        lhsT=weffs[kt][:, dt_i * 128:(dt_i + 1) * 128],
                        rhs=toks[kt][:, mi * MCHUNK:(mi + 1) * MCHUNK],
                        start=(kt == 0), stop=(kt == KT - 1))
        for mi in range(NPH_MI):
            for dt_i in range(DT):
                evict(psA[mi * DT + dt_i], dt_i,
                      slice(mi * MCHUNK, (mi + 1) * MCHUNK))

        # phase B: kt-outer with all 8 banks live, so the late-gated tokB
        # chunks are consumed in arrival order and never stall the PE.
        psB = [psum_pool.tile([128, MCHUNK], f32, name="ps", tag="ps")
               for _ in range(NPH_MI * DT)]
        for kt in range(KT):
            for mi in range(NPH_MI):
                for dt_i in range(DT):
                    m0 = HM + mi * MCHUNK
                    nc.tensor.matmul(
                        psB[mi * DT + dt_i][:],
                        lhsT=weffs[kt][:, dt_i * 128:(dt_i + 1) * 128],
                        rhs=toks[kt][:, m0:m0 + MCHUNK],
                        start=(kt == 0), stop=(kt == KT - 1))
        for mi in range(NPH_MI):
            for dt_i in range(DT):
                evict(psB[mi * DT + dt_i], dt_i,
                      slice(HM + mi * MCHUNK, HM + (mi + 1) * MCHUNK))

    nc.compile()
    return nc


# ---------------- cc mode: cooperative fold + AllGather ----------------
# Each core folds 128 rows of W_eff from its 2.5MB W row-slice, the 8
# partial (128, 1024) results are AllGathered into the full (1024, 1024)
# W_eff, and each core then multiplies its own 1024-token shard against
# it. Per-core HBM traffic: 2.5 (W) + 2 (tok) + 0.25 + 2 (cc) + 2 (out)
# ~= 8.75MB vs 17.8MB for the f16 shard-by-feature layout.
CC_MS = BN // NCORES            # 1024 tokens per core
CC_MCHUNK = 512                 # moving free dim per matmul
CC_NMI = CC_MS // CC_MCHUNK     # 2 m-waves
CC_DT = D_MODEL // 128          # 8 output d2 tiles (full feature dim)
CC_PREWARM = 60                 # PE warm dummies while fold+gather runs


def _build_cc() -> bass.Bass:
    f16 = mybir.dt.float16
    f32 = mybir.dt.float32

    nc = bacc.Bacc("TRN2", target_bir_lowering=False, debug=False,
                   num_devices=NCORES)
    tokT = nc.dram_tensor("tokT", [128, KT * CC_MS], f16,
                          kind="ExternalInput").ap()
    wsl = nc.dram_tensor("wsl", [128, BULK_DIM * D_MODEL], f16,
                         kind="ExternalInput").ap()
    bsl = nc.dram_tensor("bsl", [128, BULK_DIM * CC_DT], f32,
                         kind="ExternalInput").ap()
    outT = nc.dram_tensor("outT", [D_MODEL, CC_MS], f16,
                          kind="ExternalOutput").ap()

    with tile.TileContext(nc) as tc, ExitStack() as ctx:
        sb = ctx.enter_context(tc.tile_pool(name="sb", bufs=1))
        out_pool = ctx.enter_context(tc.tile_pool(name="osb", bufs=4))
        psum_pool = ctx.enter_context(
            tc.tile_pool(name="psum", bufs=8, space="PSUM"))
        dram = ctx.enter_context(tc.tile_pool(name="dram", bufs=1,
                                              space="DRAM"))

        mult = mybir.AluOpType.mult
        add = mybir.AluOpType.add

        # PE-warm operands (zeros). memset f32 then rounding-copy to f16.
        zf = sb.tile([128, 512], f32)
        nc.vector.memset(zf[:], 0.0)
        zmm = sb.tile([128, 128], f16)
        nc.scalar.copy(zmm[:], zf[:, 0:128])
        zrhs = sb.tile([128, 512], f16)
        nc.scalar.copy(zrhs[:], zf[:])

        # ---- load W row-slice (sync queue, ahead of tokens) ----
        wr = sb.tile([128, BULK_DIM * D_MODEL], f16)
        NCH = 5  # 2 k-groups per chunk
        for ch in range(NCH):
            csl = slice(ch * 2 * D_MODEL, (ch + 1) * 2 * D_MODEL)
            nc.sync.dma_start(wr[:, csl], wsl[:, csl])

        # tokens: 2 chunks of 4 k-tiles each, behind W on the same queue
        tok = sb.tile([128, KT * CC_MS], f16)
        half = KT * CC_MS // 2
        nc.sync.dma_start(tok[:, 0:half], tokT[:, 0:half])
        nc.sync.dma_start(tok[:, half:], tokT[:, half:])

        # ---- fold W_eff rows on DVE (chases the W DMA chunks) ----
        we = sb.tile([128, D_MODEL], f32)
        nc.vector.tensor_scalar_mul(we[:], wr[:, 0:D_MODEL], W_COEF[0])
        for j in range(1, BULK_DIM - 1):
            nc.vector.scalar_tensor_tensor(
                we[:], wr[:, j * D_MODEL:(j + 1) * D_MODEL], W_COEF[j],
                we[:], mult, add)
        wc = sb.tile([128, D_MODEL], f16)
        j = BULK_DIM - 1
        nc.vector.scalar_tensor_tensor(
            wc[:], wr[:, j * D_MODEL:(j + 1) * D_MODEL], W_COEF[j],
            we[:], mult, add)

        # ---- share the fold: bounce to DRAM, AllGather over 8 cores ----
        cc_in = dram.tile([128, D_MODEL], f16)
        cc_out = dram.tile([NCORES * 128, D_MODEL], f16, addr_space="Shared")
        nc.gpsimd.dma_start(cc_in[:], wc[:])
        nc.gpsimd.collective_compute(
            "AllGather", mybir.AluOpType.bypass,
            replica_groups=[list(range(NCORES))],
            ins=[cc_in.opt()], outs=[cc_out.opt()])

        # read the full W_eff back (scalar queue; fires as CC completes)
        weff = sb.tile([128, KT * D_MODEL], f16)
        for kt in range(KT):
            nc.scalar.dma_start(
                weff[:, kt * D_MODEL:(kt + 1) * D_MODEL],
                cc_out[kt * 128:(kt + 1) * 128, :])

        # ---- fold bias: be[p, j] = sum_k w_k bsl[p, k*8+j] ----
        bt = sb.tile([128, BULK_DIM * CC_DT], f32)
        nc.sync.dma_start(bt[:], bsl[:])
        be = sb.tile([128, CC_DT], f32)
        nc.vector.tensor_scalar_mul(be[:], bt[:, 0:CC_DT], W_COEF[0])
        for j in range(1, BULK_DIM):
            nc.vector.scalar_tensor_tensor(
                be[:], bt[:, j * CC_DT:(j + 1) * CC_DT], W_COEF[j], be[:],
                mult, add)

        # ---- matmuls: kt-outer per m-wave, chasing the gather ----
        ps0 = [psum_pool.tile([128, CC_MCHUNK], f32, name="ps", tag="ps")
               for _ in range(CC_DT)]
        for _ in range(CC_PREWARM):
            nc.tensor.matmul(ps0[0][:], lhsT=zmm[:], rhs=zrhs[:],
                             start=False, stop=False)

        def evict(ps, d2t, mi):
            ot = out_pool.tile([128, CC_MCHUNK], f16, name="ot", tag="ot")
            msl = slice(mi * CC_MCHUNK, (mi + 1) * CC_MCHUNK)
            if d2t % 2 == 0:
                nc.scalar.add(ot[:], ps[:], be[:, d2t:d2t + 1])
            else:
                nc.vector.tensor_scalar_add(ot[:], ps[:], be[:, d2t:d2t + 1])
            nc.gpsimd.dma_start(outT[d2t * 128:(d2t + 1) * 128, msl], ot[:])

        for mi in range(CC_NMI):
            psw = ps0 if mi == 0 else [
                psum_pool.tile([128, CC_MCHUNK], f32, name="ps", tag="ps")
                for _ in range(CC_DT)]
            msl = slice(mi * CC_MCHUNK, (mi + 1) * CC_MCHUNK)
            for kt in range(KT):
                for d2t in range(CC_DT):
                    nc.tensor.matmul(
                        psw[d2t][:],
                        lhsT=weff[:, kt * D_MODEL + d2t * 128:
                                  kt * D_MODEL + (d2t + 1) * 128],
                        rhs=tok[:, kt * CC_MS + mi * CC_MCHUNK:
                                kt * CC_MS + (mi + 1) * CC_MCHUNK],
                        start=(kt == 0), stop=(kt == KT - 1))
            for d2t in range(CC_DT):
                evict(psw[d2t], d2t, mi)

    nc.compile()
    return nc


def _make_in_maps_cc(boundary_tokens, W_b2b, b_b2b):
    tok16 = np.asarray(boundary_tokens, np.float32).reshape(
        BN, D_MODEL).astype(np.float16)
    # tok_wide[c][p, kt*1024 + m] = tok16[c*1024 + m, kt*128 + p]
    tw = tok16.reshape(NCORES, CC_MS, KT, 128).transpose(0, 3, 2, 1)
    W16 = np.asarray(W_b2b, np.float32).astype(np.float16)
    b = np.asarray(b_b2b, np.float32).reshape(BULK_DIM, CC_DT, 128)
    bsl = np.ascontiguousarray(
        b.transpose(2, 0, 1).reshape(128, BULK_DIM * CC_DT))
    in_maps = []
    for c in range(NCORES):
        in_maps.append({
            "tokT": np.ascontiguousarray(tw[c].reshape(128, KT * CC_MS)),
            "wsl": np.ascontiguousarray(W16[c * 128:(c + 1) * 128, :]),
            "bsl": bsl,
        })
    return in_maps


def _assemble_cc(results):
    out = np.empty((BN, D_MODEL), dtype=np.float32)
    for c in range(NCORES):
        out[c * CC_MS:(c + 1) * CC_MS, :] = results[c]["outT"].T
    return out.reshape(B, N, D_MODEL)


def _build(mode: str) -> bass.Bass:
    f32 = mybir.dt.float32
    bf16 = mybir.dt.bfloat16
    wire_dt = mybir.dt.float16 if mode in ("f16",) else f32

    nc = bacc.Bacc("TRN2", target_bir_lowering=False, debug=False,
                   num_devices=NCORES)
    tokT = nc.dram_tensor("tokT", [D_MODEL, MS], wire_dt,
                          kind="ExternalInput").ap()
    wsl = nc.dram_tensor("wsl", [D_MODEL, BULK_DIM * DS], wire_dt,
                         kind="ExternalInput").ap()
    bsl = nc.dram_tensor("bsl", [DS, BULK_DIM], f32, kind="ExternalInput").ap()
    outT = nc.dram_tensor("outT", [DS, MS], f32, kind="ExternalOutput").ap()

    with tile.TileContext(nc) as tc, ExitStack() as ctx:
        wraw_pool = ctx.enter_context(
            tc.tile_pool(name="wraw",
                         bufs=KT if mode in ("f16",) else 2))
        weff_pool = ctx.enter_context(tc.tile_pool(name="weff", bufs=KT))
        tok_pool = ctx.enter_context(tc.tile_pool(name="tok", bufs=KT))
        bias_pool = ctx.enter_context(tc.tile_pool(name="bias", bufs=2 * DT))
        psum_pool = ctx.enter_context(
            tc.tile_pool(name="psum", bufs=8, space="PSUM"))
        out_pool = ctx.enter_context(tc.tile_pool(name="osb", bufs=4))
        weffc_pool = None
        if mode != "f32":
            weffc_pool = ctx.enter_context(tc.tile_pool(name="weffc", bufs=KT))

        mult = mybir.AluOpType.mult
        add = mybir.AluOpType.add
        mm_dtype = {"bf16": bf16, "f32r": mybir.dt.float32r, "f32": f32,
                    "f16": mybir.dt.float16}[mode]

        # zero operands for PE-warming no-op matmuls (memset can't write f32r
        # directly; produce via a rounding copy). zrhs is independent of any
        # input DMA so warm-up can start immediately.
        zf = bias_pool.tile([128, 512], f32, tag="zf")
        nc.vector.memset(zf[:], 0.0)
        # casts on ACT (idle early) so they don't delay the DVE fold chains
        # (ACT Copy with an f32r out dtype is unverified -> DVE for f32r)
        zcast = nc.scalar if mode == "f16" else nc.vector
        zmm = bias_pool.tile([128, 128], mm_dtype, tag="zmm")
        zcast.copy(zmm[:], zf[:, 0:128]) if mode == "f16" else \
            nc.vector.tensor_copy(zmm[:], zf[:, 0:128])
        zrhs = bias_pool.tile([128, 512], mm_dtype, tag="zrhs")
        zcast.copy(zrhs[:], zf[:]) if mode == "f16" else \
            nc.vector.tensor_copy(zrhs[:], zf[:])

        # ---- per k-tile: load W slice, fold W_eff, load tokens ----
        toks = []
        weffs = []
        for kt in range(KT):
            ksl = slice(kt * 128, (kt + 1) * 128)
            wr = wraw_pool.tile([128, BULK_DIM * DS], wire_dt)
            if mode in ("f16",):
                # split columns so the fold chain (j ascending) starts as
                # soon as the first half lands (subtile deps)
                hw = BULK_DIM * DS // 2
                nc.scalar.dma_start(wr[:, 0:hw], wsl[ksl, 0:hw])
                nc.scalar.dma_start(wr[:, hw:], wsl[ksl, hw:])
            else:
                nc.gpsimd.dma_start(wr[:], wsl[ksl, :])

            tk = tok_pool.tile([128, MS], mm_dtype)
            if mode in ("f16",):
                # no cast needed -> HWDGE queue, decoupled from W-load waits
                nc.sync.dma_start(tk[:], tokT[ksl, :])
            else:
                nc.gpsimd.dma_start(tk[:], tokT[ksl, :])  # SWDGE rounding cast
            toks.append(tk)

            fold = nc.vector
            we = weff_pool.tile([128, DS], f32)
            fold.tensor_scalar_mul(we[:], wr[:, 0:DS], W_COEF[0])
            for j in range(1, BULK_DIM - 1):
                fold.scalar_tensor_tensor(
                    we[:], wr[:, j * DS:(j + 1) * DS], W_COEF[j], we[:],
                    mult, add)
            j = BULK_DIM - 1
            if mode == "f32":
                fold.scalar_tensor_tensor(
                    we[:], wr[:, j * DS:(j + 1) * DS], W_COEF[j], we[:],
                    mult, add)
                weffs.append(we)
            else:
                # final fold step writes the matmul dtype directly (rounds)
                wc = weffc_pool.tile([128, DS], mm_dtype)
                fold.scalar_tensor_tensor(
                    wc[:], wr[:, j * DS:(j + 1) * DS], W_COEF[j], we[:],
                    mult, add)
                weffs.append(wc)

        # ---- fold bias: beff[d] = sum_j w_j b[j*D + d], per-partition ----
        biases = []
        bfold = nc.vector
        for dt_i in range(DT):
            bt = bias_pool.tile([128, BULK_DIM], f32, tag="bt")
            nc.sync.dma_start(bt[:], bsl[dt_i * 128:(dt_i + 1) * 128, :])
            be = bias_pool.tile([128, 1], f32, tag="be")
            bfold.tensor_scalar_mul(be[:], bt[:, 0:1], W_COEF[0])
            for j in range(1, BULK_DIM):
                bfold.scalar_tensor_tensor(
                    be[:], bt[:, j:j + 1], W_COEF[j], be[:], mult, add)
            biases.append(be)

        # ---- matmul: kt-outer in 2 halves (8 live psum banks each) ----
        # Between k-steps of the first half the PE is supply-gated on DMA;
        # zero-weight no-op matmuls keep its HAM clock at 8/8 (idle >3.4us
        # re-throttles the PE to 1.2 GHz).
        n_dummy = 4 if mode != "f32" else 0
        n_prewarm = 40 if mode != "f32" else 0
        half_mi = NMI // 2

        def evict(ps, dt_i, msl):
            ot = out_pool.tile([128, MCHUNK], f32, name="ot", tag="ot")
            if dt_i == 0:
                nc.scalar.add(ot[:], ps[:], biases[dt_i][:])
                nc.scalar.dma_start(
                    outT[dt_i * 128:(dt_i + 1) * 128, msl], ot[:])
            else:
                nc.vector.tensor_scalar_add(
                    ot[:], ps[:], biases[dt_i][:, 0:1])
                nc.sync.dma_start(
                    outT[dt_i * 128:(dt_i + 1) * 128, msl], ot[:])

        # half 0: kt-outer (matmuls chase the incoming DMA stream)
        psums = [[psum_pool.tile([128, MCHUNK], f32, name="ps", tag="ps")
                  for _ in range(DT)] for _ in range(half_mi)]
        # warm the PE's HAM clock before the first real matmul; these touch
        # only memset tiles, so they run during the load phase
        for _ in range(n_prewarm):
            nc.tensor.matmul(psums[0][0][:], lhsT=zmm[:], rhs=zrhs[:],
                             start=False, stop=False)
        for kt in range(KT):
            for mi_l in range(half_mi):
                msl = slice(mi_l * MCHUNK, (mi_l + 1) * MCHUNK)
                for dt_i in range(DT):
                    nc.tensor.matmul(
                        psums[mi_l][dt_i][:],
                        lhsT=weffs[kt][:, dt_i * 128:(dt_i + 1) * 128],
                        rhs=toks[kt][:, msl],
                        start=(kt == 0), stop=(kt == KT - 1))
            if kt < KT - 1:
                for _ in range(n_dummy):
                    nc.tensor.matmul(psums[0][0][:], lhsT=zmm[:], rhs=zrhs[:],
                                     start=False, stop=False)
        for mi_l in range(half_mi):
            msl = slice(mi_l * MCHUNK, (mi_l + 1) * MCHUNK)
            for dt_i in range(DT):
                evict(psums[mi_l][dt_i], dt_i, msl)

        # half 1: everything is SBUF-resident by now, so go psum-outer --
        # each output group evicts right after its 8 matmuls, overlapping
        # the remaining groups' matmuls instead of trailing them all
        for mi_l in range(half_mi):
            mi = half_mi + mi_l
            msl = slice(mi * MCHUNK, (mi + 1) * MCHUNK)
            for dt_i in range(DT):
                ps = psum_pool.tile([128, MCHUNK], f32, name="ps", tag="ps")
                for kt in range(KT):
                    nc.tensor.matmul(
                        ps[:],
                        lhsT=weffs[kt][:, dt_i * 128:(dt_i + 1) * 128],
                        rhs=toks[kt][:, msl],
                        start=(kt == 0), stop=(kt == KT - 1))
                evict(ps, dt_i, msl)

    nc.compile()
    return nc


_BUILDERS = {"cc": _build_cc, "v2": _build_v2, "v3": _build_v3}


def _get_nc(mode: str) -> bass.Bass:
    if mode not in _BUILD_CACHE:
        _BUILD_CACHE[mode] = _BUILDERS.get(mode, lambda: _build(mode))()
    return _BUILD_CACHE[mode]


def _make_in_maps(boundary_tokens, W_b2b, b_b2b, mode):
    wire = np.float16 if mode == "f16" else np.float32
    tok = np.ascontiguousarray(
        np.asarray(boundary_tokens, dtype=np.float32)
        .reshape(BN, D_MODEL).T.astype(wire))
    W = np.asarray(W_b2b, dtype=np.float32).astype(wire).reshape(
        D_MODEL, BULK_DIM, D_MODEL)
    b = np.asarray(b_b2b, dtype=np.float32).reshape(BULK_DIM, D_MODEL)
    in_maps = []
    for c in range(NCORES):
        f, t = divmod(c, T_SHARDS)
        dsl = slice(f * DS, (f + 1) * DS)
        in_maps.append({
            "tokT": np.ascontiguousarray(tok[:, t * MS:(t + 1) * MS]),
            "wsl": np.ascontiguousarray(
                W[:, :, dsl].reshape(D_MODEL, BULK_DIM * DS)),
            "bsl": np.ascontiguousarray(b[:, dsl].T),
        })
    return in_maps


def _assemble(results):
    out = np.empty((BN, D_MODEL), dtype=np.float32)
    for c in range(NCORES):
        f, t = divmod(c, T_SHARDS)
        out[t * MS:(t + 1) * MS, f * DS:(f + 1) * DS] = results[c]["outT"].T
    return out.reshape(B, N, D_MODEL)


def run(boundary_tokens, W_b2b, b_b2b, mode=None, **spmd_kwargs):
    mode = mode or MODE
    nc = _get_nc(mode)
    if mode == "cc":
        in_maps = _make_in_maps_cc(boundary_tokens, W_b2b, b_b2b)
    elif mode in ("v2", "v3"):
        in_maps = _make_in_maps_v2(boundary_tokens, W_b2b, b_b2b,
                                   wsl3d=(mode == "v3"))
    else:
        in_maps = _make_in_maps(boundary_tokens, W_b2b, b_b2b, mode)
    res = run_bass_kernel_spmd(nc, in_maps, list(range(NCORES)), **spmd_kwargs)
    if mode == "cc":
        out = _assemble_cc(res.results)
    elif mode in ("v2", "v3"):
        out = _assemble_v2(res.results)
    else:
        out = _assemble(res.results)
    return out, res


def kernel(boundary_tokens, W_b2b, b_b2b):
    out, _ = run(boundary_tokens, W_b2b, b_b2b)
    return out



# revision 25
# speedup vs baseline: 1.0661x; 1.0661x over previous
"""Trainium2 Bass kernel for nn_BulkSpaceGenerator.

Math: the fast-marching scan g_k = g_{k-1} + (1/(k+1))(c_k - g_{k-1}) starting
from c_0 yields the running mean g_k = mean(c_0..c_k); the mean over k of those
is sum_j w_j c_j with w_j = (1/K)(H_K - H_j) (harmonic numbers). Since
c_j = tokens @ W[:, j*D:(j+1)*D] + b[j*D:(j+1)*D], the whole module is

    out = tokens @ W_eff + b_eff,   W_eff = sum_j w_j W_j,  b_eff = sum_j w_j b_j

The kernel folds W -> W_eff on-device (DVE) and runs the (8192x1024)@(1024x1024)
matmul on the PE array, sharded over 8 cores as 4 feature-shards x 2
token-shards (minimizes per-core HBM traffic: W_slice + tokens/2 + out/8).

Layout per core (f in 0..3, t in 0..1, core = f*2 + t):
  tokT : (1024, 4096) f32  -- tokens^T slice, columns t*4096:(t+1)*4096
  wsl  : (1024, 2560) f32  -- W[:, j*1024 + f*256 : j*1024 + (f+1)*256], j-major
  bsl  : (256, 10)    f32  -- b[j*1024 + f*256 + d] transposed to (d, j)
  outT : (256, 4096)  f32  -- out^T slice (host reassembles full (4,2048,1024))
"""

import os
from contextlib import ExitStack

import numpy as np

import concourse.bass as bass
import concourse.tile as tile
from concourse import bacc, mybir
from concourse.bass_utils import run_bass_kernel_spmd

D_MODEL = 1024
BULK_DIM = 10
B, N = 4, 2048
BN = B * N                     # 8192 tokens
NCORES = 8
F_SHARDS = 4                   # feature shards (d dimension)
T_SHARDS = 2                   # token shards
DS = D_MODEL // F_SHARDS       # 256 output features per core
MS = BN // T_SHARDS            # 4096 tokens per core
KT = D_MODEL // 128            # 8 contraction k-tiles
DT = DS // 128                 # 2 output d-tiles of 128 per core
MCHUNK = 512                   # moving free dim per matmul
NMI = MS // MCHUNK             # 8 m-chunks per core

# w_j = (1/K) * (H_K - H_j), H_j = sum_{i=1..j} 1/i
_H = np.cumsum(1.0 / np.arange(1, BULK_DIM + 1))
W_COEF = ((_H[-1] - np.concatenate([[0.0], _H[:-1]])) / BULK_DIM).tolist()

# mode: "f32r" | "f32" | "bf16" keep f32 inputs on the wire; "f16" ships
# tokens and W as fp16 (half the load bytes, ~3.6e-4 rel err vs 1.5e-4 f32r)
# "cc": 8-way token shard + cooperative W_eff fold shared via AllGather
#       (dead: collectives have ~85us fixed cost under axon NRT)
# "v2": f16 wire + f16 out, kt-outer streaming in two m-phases
MODE = os.environ.get("BULK_KERNEL_MODE", "v2")

_BUILD_CACHE = {}

# ---------------- v2: f16 wire, kt-outer streaming, f16 out ----------------
# Same 4 feature-shards x 2 token-shards as f16 mode, but:
#  - DMA order per kt: W rows-chunk then phase-A token chunk, so the PE
#    starts accumulating at ~3us and chases the incoming stream
#  - W_eff fold runs in f16 (2x DVE rate); accuracy cost ~1e-4, fine
#  - psum is 8 banks of (128, 512): phase A = m 0..2047 kt-outer,
#    phase B = m 2048..4095 kt-outer once A's banks evict
#  - output ships f16 (halves store traffic; host upcasts)


def _build_v2() -> bass.Bass:
    f16 = mybir.dt.float16
    f32 = mybir.dt.float32

    nc = bacc.Bacc("TRN2", target_bir_lowering=False, debug=False,
                   num_devices=NCORES)
    tokT = nc.dram_tensor("tokT", [D_MODEL, MS], f16,
                          kind="ExternalInput").ap()
    wsl = nc.dram_tensor("wsl", [D_MODEL, BULK_DIM * DS], f16,
                         kind="ExternalInput").ap()
    bsl = nc.dram_tensor("bsl", [DS, BULK_DIM], f32, kind="ExternalInput").ap()
    outT = nc.dram_tensor("outT", [DS, MS], f16, kind="ExternalOutput").ap()

    HM = MS // 2                # 2048 tokens per phase
    NPH_MI = HM // MCHUNK       # 4 m-chunks per phase

    with tile.TileContext(nc) as tc, ExitStack() as ctx:
        sb = ctx.enter_context(tc.tile_pool(name="sb", bufs=1))
        out_pool = ctx.enter_context(tc.tile_pool(name="osb", bufs=4))
        psum_pool = ctx.enter_context(
            tc.tile_pool(name="psum", bufs=8, space="PSUM"))

        mult = mybir.AluOpType.mult
        add = mybir.AluOpType.add

        # PE warm operands
        zf = sb.tile([128, 512], f32)
        nc.vector.memset(zf[:], 0.0)
        zmm = sb.tile([128, 128], f16)
        nc.scalar.copy(zmm[:], zf[:, 0:128])
        zrhs = sb.tile([128, 512], f16)
        nc.scalar.copy(zrhs[:], zf[:])

        # ---- input stream (sync queue, program order == priority) ----
        wrs, toks = [], []
        for kt in range(KT):
            ksl = slice(kt * 128, (kt + 1) * 128)
            wr = sb.tile([128, BULK_DIM * DS], f16, name=f"wr{kt}")
            nc.sync.dma_start(wr[:], wsl[ksl, :])
            wrs.append(wr)
            tk = sb.tile([128, MS], f16, name=f"tk{kt}")
            nc.sync.dma_start(tk[:, 0:HM], tokT[ksl, 0:HM])
            toks.append(tk)
        for kt in range(KT):
            ksl = slice(kt * 128, (kt + 1) * 128)
            nc.sync.dma_start(toks[kt][:, HM:], tokT[ksl, HM:])

        # ---- fold W_eff per kt on DVE, all-f16 (chases the W DMAs) ----
        weffs = []
        for kt in range(KT):
            we = sb.tile([128, DS], f16, name=f"we{kt}")
            nc.vector.tensor_scalar_mul(we[:], wrs[kt][:, 0:DS], W_COEF[0])
            for j in range(1, BULK_DIM):
                nc.vector.scalar_tensor_tensor(
                    we[:], wrs[kt][:, j * DS:(j + 1) * DS], W_COEF[j],
                    we[:], mult, add)
            weffs.append(we)

        # ---- bias fold (f32, tiny) ----
        biases = []
        for dt_i in range(DT):
            bt = sb.tile([128, BULK_DIM], f32, name=f"bt{dt_i}")
            nc.scalar.dma_start(bt[:], bsl[dt_i * 128:(dt_i + 1) * 128, :])
            be = sb.tile([128, 1], f32, name=f"be{dt_i}")
            nc.vector.tensor_scalar_mul(be[:], bt[:, 0:1], W_COEF[0])
            for j in range(1, BULK_DIM):
                nc.vector.scalar_tensor_tensor(
                    be[:], bt[:, j:j + 1], W_COEF[j], be[:], mult, add)
            biases.append(be)

        def evict(ps, dt_i, msl):
            ot = out_pool.tile([128, MCHUNK], f16, name="ot", tag="ot")
            if dt_i == 0:
                nc.scalar.add(ot[:], ps[:], biases[dt_i][:])
            else:
                nc.vector.tensor_scalar_add(ot[:], ps[:], biases[dt_i][:, 0:1])
            nc.gpsimd.dma_start(outT[dt_i * 128:(dt_i + 1) * 128, msl], ot[:])

        # keep the PE HAM clock warm until the first weff lands
        ps_warm = psum_pool.tile([128, MCHUNK], f32, name="ps", tag="ps")
        for _ in range(16):
            nc.tensor.matmul(ps_warm[:], lhsT=zmm[:], rhs=zrhs[:],
                             start=False, stop=False)

        # ---- phase A: m 0..HM, kt-outer over 8 live psum groups ----
        psA = [ps_warm] + [
            psum_pool.tile([128, MCHUNK], f32, name="ps", tag="ps")
            for _ in range(NPH_MI * DT - 1)]
        for kt in range(KT):
            for dt_i in range(DT):
                for mi in range(NPH_MI):
                    nc.tensor.matmul(
                        psA[mi * DT + dt_i][:],
                        lhsT=weffs[kt][:, dt_i * 128:(dt_i + 1) * 128],
                        rhs=toks[kt][:, mi * MCHUNK:(mi + 1) * MCHUNK],
                        start=(kt == 0), stop=(kt == KT - 1))
        for mi in range(NPH_MI):
            for dt_i in range(DT):
                evict(psA[mi * DT + dt_i], dt_i,
                      slice(mi * MCHUNK, (mi + 1) * MCHUNK))

        # ---- phase B: m HM..MS ----
        # phase B: tokens are resident by now, so run single-group
        # accumulation chains and evict each group immediately -- spreads
        # the evict+store tail across phase B instead of piling it at the
        # end of the kernel.
        for mi in range(NPH_MI):
            for dt_i in range(DT):
                ps = psum_pool.tile([128, MCHUNK], f32, name="ps", tag="ps")
                m0 = HM + mi * MCHUNK
                for kt in range(KT):
                    nc.tensor.matmul(
                        ps[:],
                        lhsT=weffs[kt][:, dt_i * 128:(dt_i + 1) * 128],
                        rhs=toks[kt][:, m0:m0 + MCHUNK],
                        start=(kt == 0), stop=(kt == KT - 1))
                evict(ps, dt_i, slice(m0, m0 + MCHUNK))

    nc.compile()
    return nc


def _make_in_maps_v2(boundary_tokens, W_b2b, b_b2b, wsl3d=False):
    tok = np.ascontiguousarray(
        np.asarray(boundary_tokens, dtype=np.float32)
        .reshape(BN, D_MODEL).T.astype(np.float16))
    W = np.asarray(W_b2b, np.float32).astype(np.float16).reshape(
        D_MODEL, BULK_DIM, D_MODEL)
    b = np.asarray(b_b2b, np.float32).reshape(BULK_DIM, D_MODEL)
    wshape = ((D_MODEL, BULK_DIM, DS) if wsl3d
              else (D_MODEL, BULK_DIM * DS))
    in_maps = []
    for c in range(NCORES):
        f, t = divmod(c, T_SHARDS)
        dsl = slice(f * DS, (f + 1) * DS)
        in_maps.append({
            "tokT": np.ascontiguousarray(tok[:, t * MS:(t + 1) * MS]),
            "wsl": np.ascontiguousarray(
                W[:, :, dsl].reshape(*wshape)),
            "bsl": np.ascontiguousarray(b[:, dsl].T),
        })
    return in_maps


def _assemble_v2(results):
    out = np.empty((BN, D_MODEL), dtype=np.float32)
    for c in range(NCORES):
        f, t = divmod(c, T_SHARDS)
        out[t * MS:(t + 1) * MS, f * DS:(f + 1) * DS] = results[c]["outT"].T
    return out.reshape(B, N, D_MODEL)


# ------- v4: v2 + fold restructured to fit the DMA arrival cadence -------
# The W_eff fold is 2 ALU ops/elem; v2's 10-step STT chain costs 4.25us
# per kt-tile on the DVE vs the 3.26us DMA arrival cadence, so it paced
# the whole kernel. v4 folds each kt as:
#   vector : tmp[0:6] = W[0:6] * C  (one big tensor_mul vs a memset
#            coefficient tile), then an in-place pairwise add tree
#   scalar : tmp[6..9] = cj*Wj      (4 ACT scale-copies)
# ~2.8us/kt on vector, ~1.6us/kt on scalar (gpsimd can't run DVE ops on
# real TRN2 - ISA check rejects Pool-engine TensorScalar).


def _build_v3() -> bass.Bass:
    f16 = mybir.dt.float16
    f32 = mybir.dt.float32

    nc = bacc.Bacc("TRN2", target_bir_lowering=False, debug=False,
                   num_devices=NCORES)
    tokT = nc.dram_tensor("tokT", [D_MODEL, MS], f16,
                          kind="ExternalInput").ap()
    wsl = nc.dram_tensor("wsl", [D_MODEL, BULK_DIM, DS], f16,
                         kind="ExternalInput").ap()
    bsl = nc.dram_tensor("bsl", [DS, BULK_DIM], f32, kind="ExternalInput").ap()
    outT = nc.dram_tensor("outT", [DS, MS], f16, kind="ExternalOutput").ap()

    HM = MS // 2
    NPH_MI = HM // MCHUNK

    with tile.TileContext(nc) as tc, ExitStack() as ctx:
        sb = ctx.enter_context(tc.tile_pool(name="sb", bufs=1))
        tmp_pool = ctx.enter_context(tc.tile_pool(name="tmp", bufs=2))
        out_pool = ctx.enter_context(tc.tile_pool(name="osb", bufs=4))
        psum_pool = ctx.enter_context(
            tc.tile_pool(name="psum", bufs=8, space="PSUM"))

        mult = mybir.AluOpType.mult
        add = mybir.AluOpType.add

        zf = sb.tile([128, 512], f32)
        nc.vector.memset(zf[:], 0.0)
        zmm = sb.tile([128, 128], f16)
        nc.scalar.copy(zmm[:], zf[:, 0:128])
        zrhs = sb.tile([128, 512], f16)
        nc.scalar.copy(zrhs[:], zf[:])

        # coefficient tile for the one-op fold multiply (j = 0..5)
        cco = sb.tile([128, 6, DS], f16)
        for j in range(6):
            nc.vector.memset(cco[:, j, :], float(W_COEF[j]))

        # W chunks run one step ahead of the phase-A token chunks so the
        # fold is never the late dependency of a kt batch.
        wrs = [sb.tile([128, BULK_DIM, DS], f16, name=f"wr{kt}")
               for kt in range(KT)]
        toks = [sb.tile([128, MS], f16, name=f"tk{kt}") for kt in range(KT)]

        def ksl(kt):
            return slice(kt * 128, (kt + 1) * 128)

        nc.sync.dma_start(wrs[0][:], wsl[ksl(0), :, :])
        for kt in range(1, KT):
            nc.sync.dma_start(wrs[kt][:], wsl[ksl(kt), :, :])
            nc.sync.dma_start(toks[kt - 1][:, 0:HM], tokT[ksl(kt - 1), 0:HM])
        nc.sync.dma_start(toks[KT - 1][:, 0:HM], tokT[ksl(KT - 1), 0:HM])

        for kt in range(KT):
            nc.sync.dma_start(toks[kt][:, HM:], tokT[ksl(kt), HM:])

        # ---- fold: big-op multiply + pairwise add tree ----
        weffs = []
        for kt in range(KT):
            wr = wrs[kt]
            tmp = tmp_pool.tile([128, BULK_DIM, DS], f16, name="tmp",
                                tag="tmp")
            nc.vector.tensor_mul(tmp[:, 0:6, :], wr[:, 0:6, :], cco[:])
            for j in range(6, BULK_DIM):
                nc.scalar.mul(tmp[:, j, :], wr[:, j, :], W_COEF[j])
            nc.vector.tensor_add(tmp[:, 0:5, :], tmp[:, 0:5, :],
                                 tmp[:, 5:10, :])
            nc.vector.tensor_add(tmp[:, 0:2, :], tmp[:, 0:2, :],
                                 tmp[:, 2:4, :])
            nc.vector.tensor_add(tmp[:, 0, :], tmp[:, 0, :], tmp[:, 1, :])
            we = sb.tile([128, DS], f16, name=f"we{kt}")
            nc.vector.tensor_add(we[:], tmp[:, 0, :], tmp[:, 4, :])
            weffs.append(we)

        biases = []
        for dt_i in range(DT):
            bt = sb.tile([128, BULK_DIM], f32, name=f"bt{dt_i}")
            nc.scalar.dma_start(bt[:], bsl[dt_i * 128:(dt_i + 1) * 128, :])
            be = sb.tile([128, 1], f32, name=f"be{dt_i}")
            nc.vector.tensor_scalar_mul(be[:], bt[:, 0:1], W_COEF[0])
            for j in range(1, BULK_DIM):
                nc.vector.scalar_tensor_tensor(
                    be[:], bt[:, j:j + 1], W_COEF[j], be[:], mult, add)
            biases.append(be)

        def evict(ps, dt_i, msl):
            ot = out_pool.tile([128, MCHUNK], f16, name="ot", tag="ot")
            if dt_i == 0:
                nc.scalar.add(ot[:], ps[:], biases[dt_i][:])
            else:
                nc.vector.tensor_scalar_add(ot[:], ps[:], biases[dt_i][:, 0:1])
            nc.sync.dma_start(outT[dt_i * 128:(dt_i + 1) * 128, msl], ot[:])

        ps_warm = psum_pool.tile([128, MCHUNK], f32, name="ps", tag="ps")
        for _ in range(24):
            nc.tensor.matmul(ps_warm[:], lhsT=zmm[:], rhs=zrhs[:],
                             start=False, stop=False)

        psA = [ps_warm] + [
            psum_pool.tile([128, MCHUNK], f32, name="ps", tag="ps")
            for _ in range(NPH_MI * DT - 1)]
        for kt in range(KT):
            for dt_i in range(DT):
                for mi in range(NPH_MI):
                    nc.tensor.matmul(
                        psA[mi * DT + dt_i][:],
                        lhsT=weffs[kt][:, dt_i * 128:(dt_i + 1) * 128],
                        rhs=toks[kt][:, mi * MCHUNK:(mi + 1) * MCHUNK],
                        start=(kt == 0), stop=(kt == KT - 1))
        for mi in range(NPH_MI):
            for dt_i in range(DT):
                evict(psA[mi * DT + dt_i], dt_i,
                      slice(mi * MCHUNK, (mi + 1) * MCHUNK))

        # phase B: tokens are resident by now, so run single-group
        # accumulation chains and evict each group immediately -- spreads
        # the evict+store tail across phase B instead of piling it at the
        # end of the kernel.
        for mi in range(NPH_MI):
            for dt_i in range(DT):
                ps = psum_pool.tile([128, MCHUNK], f32, name="ps", tag="ps")
                m0 = HM + mi * MCHUNK
                for kt in range(KT):
                    nc.tensor.matmul(
                        ps[:],
                        lhsT=weffs[kt][:, dt_i * 128:(dt_i + 1) * 128],
                        rhs=toks[kt][:, m0:m0 + MCHUNK],
                        start=(kt == 0), stop=(kt == KT - 1))
                evict(ps, dt_i, slice(m0, m0 + MCHUNK))

    nc.compile()
    return nc


# ---------------- cc mode: cooperative fold + AllGather ----------------
# Each core folds 128 rows of W_eff from its 2.5MB W row-slice, the 8
# partial (128, 1024) results are AllGathered into the full (1024, 1024)
# W_eff, and each core then multiplies its own 1024-token shard against
# it. Per-core HBM traffic: 2.5 (W) + 2 (tok) + 0.25 + 2 (cc) + 2 (out)
# ~= 8.75MB vs 17.8MB for the f16 shard-by-feature layout.
CC_MS = BN // NCORES            # 1024 tokens per core
CC_MCHUNK = 512                 # moving free dim per matmul
CC_NMI = CC_MS // CC_MCHUNK     # 2 m-waves
CC_DT = D_MODEL // 128          # 8 output d2 tiles (full feature dim)
CC_PREWARM = 60                 # PE warm dummies while fold+gather runs


def _build_cc() -> bass.Bass:
    f16 = mybir.dt.float16
    f32 = mybir.dt.float32

    nc = bacc.Bacc("TRN2", target_bir_lowering=False, debug=False,
                   num_devices=NCORES)
    tokT = nc.dram_tensor("tokT", [128, KT * CC_MS], f16,
                          kind="ExternalInput").ap()
    wsl = nc.dram_tensor("wsl", [128, BULK_DIM * D_MODEL], f16,
                         kind="ExternalInput").ap()
    bsl = nc.dram_tensor("bsl", [128, BULK_DIM * CC_DT], f32,
                         kind="ExternalInput").ap()
    outT = nc.dram_tensor("outT", [D_MODEL, CC_MS], f16,
                          kind="ExternalOutput").ap()

    with tile.TileContext(nc) as tc, ExitStack() as ctx:
        sb = ctx.enter_context(tc.tile_pool(name="sb", bufs=1))
        out_pool = ctx.enter_context(tc.tile_pool(name="osb", bufs=4))
        psum_pool = ctx.enter_context(
            tc.tile_pool(name="psum", bufs=8, space="PSUM"))
        dram = ctx.enter_context(tc.tile_pool(name="dram", bufs=1,
                                              space="DRAM"))

        mult = mybir.AluOpType.mult
        add = mybir.AluOpType.add

        # PE-warm operands (zeros). memset f32 then rounding-copy to f16.
        zf = sb.tile([128, 512], f32)
        nc.vector.memset(zf[:], 0.0)
        zmm = sb.tile([128, 128], f16)
        nc.scalar.copy(zmm[:], zf[:, 0:128])
        zrhs = sb.tile([128, 512], f16)
        nc.scalar.copy(zrhs[:], zf[:])

        # ---- load W row-slice (sync queue, ahead of tokens) ----
        wr = sb.tile([128, BULK_DIM * D_MODEL], f16)
        NCH = 5  # 2 k-groups per chunk
        for ch in range(NCH):
            csl = slice(ch * 2 * D_MODEL, (ch + 1) * 2 * D_MODEL)
            nc.sync.dma_start(wr[:, csl], wsl[:, csl])

        # tokens: 2 chunks of 4 k-tiles each, behind W on the same queue
        tok = sb.tile([128, KT * CC_MS], f16)
        half = KT * CC_MS // 2
        nc.sync.dma_start(tok[:, 0:half], tokT[:, 0:half])
        nc.sync.dma_start(tok[:, half:], tokT[:, half:])

        # ---- fold W_eff rows on DVE (chases the W DMA chunks) ----
        we = sb.tile([128, D_MODEL], f32)
        nc.vector.tensor_scalar_mul(we[:], wr[:, 0:D_MODEL], W_COEF[0])
        for j in range(1, BULK_DIM - 1):
            nc.vector.scalar_tensor_tensor(
                we[:], wr[:, j * D_MODEL:(j + 1) * D_MODEL], W_COEF[j],
                we[:], mult, add)
        wc = sb.tile([128, D_MODEL], f16)
        j = BULK_DIM - 1
        nc.vector.scalar_tensor_tensor(
            wc[:], wr[:, j * D_MODEL:(j + 1) * D_MODEL], W_COEF[j],
            we[:], mult, add)

        # ---- share the fold: bounce to DRAM, AllGather over 8 cores ----
        cc_in = dram.tile([128, D_MODEL], f16)
        cc_out = dram.tile([NCORES * 128, D_MODEL], f16, addr_space="Shared")
        nc.gpsimd.dma_start(cc_in[:], wc[:])
        nc.gpsimd.collective_compute(
            "AllGather", mybir.AluOpType.bypass,
            replica_groups=[list(range(NCORES))],
            ins=[cc_in.opt()], outs=[cc_out.opt()])

        # read the full W_eff back (scalar queue; fires as CC completes)
        weff = sb.tile([128, KT * D_MODEL], f16)
        for kt in range(KT):
            nc.scalar.dma_start(
                weff[:, kt * D_MODEL:(kt + 1) * D_MODEL],
                cc_out[kt * 128:(kt + 1) * 128, :])

        # ---- fold bias: be[p, j] = sum_k w_k bsl[p, k*8+j] ----
        bt = sb.tile([128, BULK_DIM * CC_DT], f32)
        nc.sync.dma_start(bt[:], bsl[:])
        be = sb.tile([128, CC_DT], f32)
        nc.vector.tensor_scalar_mul(be[:], bt[:, 0:CC_DT], W_COEF[0])
        for j in range(1, BULK_DIM):
            nc.vector.scalar_tensor_tensor(
                be[:], bt[:, j * CC_DT:(j + 1) * CC_DT], W_COEF[j], be[:],
                mult, add)

        # ---- matmuls: kt-outer per m-wave, chasing the gather ----
        ps0 = [psum_pool.tile([128, CC_MCHUNK], f32, name="ps", tag="ps")
               for _ in range(CC_DT)]
        for _ in range(CC_PREWARM):
            nc.tensor.matmul(ps0[0][:], lhsT=zmm[:], rhs=zrhs[:],
                             start=False, stop=False)

        def evict(ps, d2t, mi):
            ot = out_pool.tile([128, CC_MCHUNK], f16, name="ot", tag="ot")
            msl = slice(mi * CC_MCHUNK, (mi + 1) * CC_MCHUNK)
            if d2t % 2 == 0:
                nc.scalar.add(ot[:], ps[:], be[:, d2t:d2t + 1])
            else:
                nc.vector.tensor_scalar_add(ot[:], ps[:], be[:, d2t:d2t + 1])
            nc.gpsimd.dma_start(outT[d2t * 128:(d2t + 1) * 128, msl], ot[:])

        for mi in range(CC_NMI):
            psw = ps0 if mi == 0 else [
                psum_pool.tile([128, CC_MCHUNK], f32, name="ps", tag="ps")
                for _ in range(CC_DT)]
            msl = slice(mi * CC_MCHUNK, (mi + 1) * CC_MCHUNK)
            for kt in range(KT):
                for d2t in range(CC_DT):
                    nc.tensor.matmul(
                        psw[d2t][:],
                        lhsT=weff[:, kt * D_MODEL + d2t * 128:
                                  kt * D_MODEL + (d2t + 1) * 128],
                        rhs=tok[:, kt * CC_MS + mi * CC_MCHUNK:
                                kt * CC_MS + (mi + 1) * CC_MCHUNK],
                        start=(kt == 0), stop=(kt == KT - 1))
            for d2t in range(CC_DT):
                evict(psw[d2t], d2t, mi)

    nc.compile()
    return nc


def _make_in_maps_cc(boundary_tokens, W_b2b, b_b2b):
    tok16 = np.asarray(boundary_tokens, np.float32).reshape(
        BN, D_MODEL).astype(np.float16)
    # tok_wide[c][p, kt*1024 + m] = tok16[c*1024 + m, kt*128 + p]
    tw = tok16.reshape(NCORES, CC_MS, KT, 128).transpose(0, 3, 2, 1)
    W16 = np.asarray(W_b2b, np.float32).astype(np.float16)
    b = np.asarray(b_b2b, np.float32).reshape(BULK_DIM, CC_DT, 128)
    bsl = np.ascontiguousarray(
        b.transpose(2, 0, 1).reshape(128, BULK_DIM * CC_DT))
    in_maps = []
    for c in range(NCORES):
        in_maps.append({
            "tokT": np.ascontiguousarray(tw[c].reshape(128, KT * CC_MS)),
            "wsl": np.ascontiguousarray(W16[c * 128:(c + 1) * 128, :]),
            "bsl": bsl,
        })
    return in_maps


def _assemble_cc(results):
    out = np.empty((BN, D_MODEL), dtype=np.float32)
    for c in range(NCORES):
        out[c * CC_MS:(c + 1) * CC_MS, :] = results[c]["outT"].T
    return out.reshape(B, N, D_MODEL)


def _build(mode: str) -> bass.Bass:
    f32 = mybir.dt.float32
    bf16 = mybir.dt.bfloat16
    wire_dt = mybir.dt.float16 if mode in ("f16",) else f32

    nc = bacc.Bacc("TRN2", target_bir_lowering=False, debug=False,
                   num_devices=NCORES)
    tokT = nc.dram_tensor("tokT", [D_MODEL, MS], wire_dt,
                          kind="ExternalInput").ap()
    wsl = nc.dram_tensor("wsl", [D_MODEL, BULK_DIM * DS], wire_dt,
                         kind="ExternalInput").ap()
    bsl = nc.dram_tensor("bsl", [DS, BULK_DIM], f32, kind="ExternalInput").ap()
    outT = nc.dram_tensor("outT", [DS, MS], f32, kind="ExternalOutput").ap()

    with tile.TileContext(nc) as tc, ExitStack() as ctx:
        wraw_pool = ctx.enter_context(
            tc.tile_pool(name="wraw",
                         bufs=KT if mode in ("f16",) else 2))
        weff_pool = ctx.enter_context(tc.tile_pool(name="weff", bufs=KT))
        tok_pool = ctx.enter_context(tc.tile_pool(name="tok", bufs=KT))
        bias_pool = ctx.enter_context(tc.tile_pool(name="bias", bufs=2 * DT))
        psum_pool = ctx.enter_context(
            tc.tile_pool(name="psum", bufs=8, space="PSUM"))
        out_pool = ctx.enter_context(tc.tile_pool(name="osb", bufs=4))
        weffc_pool = None
        if mode != "f32":
            weffc_pool = ctx.enter_context(tc.tile_pool(name="weffc", bufs=KT))

        mult = mybir.AluOpType.mult
        add = mybir.AluOpType.add
        mm_dtype = {"bf16": bf16, "f32r": mybir.dt.float32r, "f32": f32,
                    "f16": mybir.dt.float16}[mode]

        # zero operands for PE-warming no-op matmuls (memset can't write f32r
        # directly; produce via a rounding copy). zrhs is independent of any
        # input DMA so warm-up can start immediately.
        zf = bias_pool.tile([128, 512], f32, tag="zf")
        nc.vector.memset(zf[:], 0.0)
        # casts on ACT (idle early) so they don't delay the DVE fold chains
        # (ACT Copy with an f32r out dtype is unverified -> DVE for f32r)
        zcast = nc.scalar if mode == "f16" else nc.vector
        zmm = bias_pool.tile([128, 128], mm_dtype, tag="zmm")
        zcast.copy(zmm[:], zf[:, 0:128]) if mode == "f16" else \
            nc.vector.tensor_copy(zmm[:], zf[:, 0:128])
        zrhs = bias_pool.tile([128, 512], mm_dtype, tag="zrhs")
        zcast.copy(zrhs[:], zf[:]) if mode == "f16" else \
            nc.vector.tensor_copy(zrhs[:], zf[:])

        # ---- per k-tile: load W slice, fold W_eff, load tokens ----
        toks = []
        weffs = []
        for kt in range(KT):
            ksl = slice(kt * 128, (kt + 1) * 128)
            wr = wraw_pool.tile([128, BULK_DIM * DS], wire_dt)
            if mode in ("f16",):
                # split columns so the fold chain (j ascending) starts as
                # soon as the first half lands (subtile deps)
                hw = BULK_DIM * DS // 2
                nc.scalar.dma_start(wr[:, 0:hw], wsl[ksl, 0:hw])
                nc.scalar.dma_start(wr[:, hw:], wsl[ksl, hw:])
            else:
                nc.gpsimd.dma_start(wr[:], wsl[ksl, :])

            tk = tok_pool.tile([128, MS], mm_dtype)
            if mode in ("f16",):
                # no cast needed -> HWDGE queue, decoupled from W-load waits
                nc.sync.dma_start(tk[:], tokT[ksl, :])
            else:
                nc.gpsimd.dma_start(tk[:], tokT[ksl, :])  # SWDGE rounding cast
            toks.append(tk)

            fold = nc.vector
            we = weff_pool.tile([128, DS], f32)
            fold.tensor_scalar_mul(we[:], wr[:, 0:DS], W_COEF[0])
            for j in range(1, BULK_DIM - 1):
                fold.scalar_tensor_tensor(
                    we[:], wr[:, j * DS:(j + 1) * DS], W_COEF[j], we[:],
                    mult, add)
            j = BULK_DIM - 1
            if mode == "f32":
                fold.scalar_tensor_tensor(
                    we[:], wr[:, j * DS:(j + 1) * DS], W_COEF[j], we[:],
                    mult, add)
                weffs.append(we)
            else:
                # final fold step writes the matmul dtype directly (rounds)
                wc = weffc_pool.tile([128, DS], mm_dtype)
                fold.scalar_tensor_tensor(
                    wc[:], wr[:, j * DS:(j + 1) * DS], W_COEF[j], we[:],
                    mult, add)
                weffs.append(wc)

        # ---- fold bias: beff[d] = sum_j w_j b[j*D + d], per-partition ----
        biases = []
        bfold = nc.vector
        for dt_i in range(DT):
            bt = bias_pool.tile([128, BULK_DIM], f32, tag="bt")
            nc.sync.dma_start(bt[:], bsl[dt_i * 128:(dt_i + 1) * 128, :])
            be = bias_pool.tile([128, 1], f32, tag="be")
            bfold.tensor_scalar_mul(be[:], bt[:, 0:1], W_COEF[0])
            for j in range(1, BULK_DIM):
                bfold.scalar_tensor_tensor(
                    be[:], bt[:, j:j + 1], W_COEF[j], be[:], mult, add)
            biases.append(be)

        # ---- matmul: kt-outer in 2 halves (8 live psum banks each) ----
        # Between k-steps of the first half the PE is supply-gated on DMA;
        # zero-weight no-op matmuls keep its HAM clock at 8/8 (idle >3.4us
        # re-throttles the PE to 1.2 GHz).
        n_dummy = 4 if mode != "f32" else 0
        n_prewarm = 40 if mode != "f32" else 0
        half_mi = NMI // 2

        def evict(ps, dt_i, msl):
            ot = out_pool.tile([128, MCHUNK], f32, name="ot", tag="ot")
            if dt_i == 0:
                nc.scalar.add(ot[:], ps[:], biases[dt_i][:])
                nc.scalar.dma_start(
                    outT[dt_i * 128:(dt_i + 1) * 128, msl], ot[:])
            else:
                nc.vector.tensor_scalar_add(
                    ot[:], ps[:], biases[dt_i][:, 0:1])
                nc.sync.dma_start(
                    outT[dt_i * 128:(dt_i + 1) * 128, msl], ot[:])

        # half 0: kt-outer (matmuls chase the incoming DMA stream)
        psums = [[psum_pool.tile([128, MCHUNK], f32, name="ps", tag="ps")
                  for _ in range(DT)] for _ in range(half_mi)]
        # warm the PE's HAM clock before the first real matmul; these touch
        # only memset tiles, so they run during the load phase
        for _ in range(n_prewarm):
            nc.tensor.matmul(psums[0][0][:], lhsT=zmm[:], rhs=zrhs[:],
                             start=False, stop=False)
        for kt in range(KT):
            for mi_l in range(half_mi):
                msl = slice(mi_l * MCHUNK, (mi_l + 1) * MCHUNK)
                for dt_i in range(DT):
                    nc.tensor.matmul(
                        psums[mi_l][dt_i][:],
                        lhsT=weffs[kt][:, dt_i * 128:(dt_i + 1) * 128],
                        rhs=toks[kt][:, msl],
                        start=(kt == 0), stop=(kt == KT - 1))
            if kt < KT - 1:
                for _ in range(n_dummy):
                    nc.tensor.matmul(psums[0][0][:], lhsT=zmm[:], rhs=zrhs[:],
                                     start=False, stop=False)
        for mi_l in range(half_mi):
            msl = slice(mi_l * MCHUNK, (mi_l + 1) * MCHUNK)
            for dt_i in range(DT):
                evict(psums[mi_l][dt_i], dt_i, msl)

        # half 1: everything is SBUF-resident by now, so go psum-outer --
        # each output group evicts right after its 8 matmuls, overlapping
        # the remaining groups' matmuls instead of trailing them all
        for mi_l in range(half_mi):
            mi = half_mi + mi_l
            msl = slice(mi * MCHUNK, (mi + 1) * MCHUNK)
            for dt_i in range(DT):
                ps = psum_pool.tile([128, MCHUNK], f32, name="ps", tag="ps")
                for kt in range(KT):
                    nc.tensor.matmul(
                        ps[:],
                        lhsT=weffs[kt][:, dt_i * 128:(dt_i + 1) * 128],
                        rhs=toks[kt][:, msl],
                        start=(kt == 0), stop=(kt == KT - 1))
                evict(ps, dt_i, msl)

    nc.compile()
    return nc


_BUILDERS = {"cc": _build_cc, "v2": _build_v2, "v3": _build_v3}


def _get_nc(mode: str) -> bass.Bass:
    if mode not in _BUILD_CACHE:
        _BUILD_CACHE[mode] = _BUILDERS.get(mode, lambda: _build(mode))()
    return _BUILD_CACHE[mode]


def _make_in_maps(boundary_tokens, W_b2b, b_b2b, mode):
    wire = np.float16 if mode == "f16" else np.float32
    tok = np.ascontiguousarray(
        np.asarray(boundary_tokens, dtype=np.float32)
        .reshape(BN, D_MODEL).T.astype(wire))
    W = np.asarray(W_b2b, dtype=np.float32).astype(wire).reshape(
        D_MODEL, BULK_DIM, D_MODEL)
    b = np.asarray(b_b2b, dtype=np.float32).reshape(BULK_DIM, D_MODEL)
    in_maps = []
    for c in range(NCORES):
        f, t = divmod(c, T_SHARDS)
        dsl = slice(f * DS, (f + 1) * DS)
        in_maps.append({
            "tokT": np.ascontiguousarray(tok[:, t * MS:(t + 1) * MS]),
            "wsl": np.ascontiguousarray(
                W[:, :, dsl].reshape(D_MODEL, BULK_DIM * DS)),
            "bsl": np.ascontiguousarray(b[:, dsl].T),
        })
    return in_maps


def _assemble(results):
    out = np.empty((BN, D_MODEL), dtype=np.float32)
    for c in range(NCORES):
        f, t = divmod(c, T_SHARDS)
        out[t * MS:(t + 1) * MS, f * DS:(f + 1) * DS] = results[c]["outT"].T
    return out.reshape(B, N, D_MODEL)


def run(boundary_tokens, W_b2b, b_b2b, mode=None, **spmd_kwargs):
    mode = mode or MODE
    nc = _get_nc(mode)
    if mode == "cc":
        in_maps = _make_in_maps_cc(boundary_tokens, W_b2b, b_b2b)
    elif mode in ("v2", "v3"):
        in_maps = _make_in_maps_v2(boundary_tokens, W_b2b, b_b2b,
                                   wsl3d=(mode == "v3"))
    else:
        in_maps = _make_in_maps(boundary_tokens, W_b2b, b_b2b, mode)
    res = run_bass_kernel_spmd(nc, in_maps, list(range(NCORES)), **spmd_kwargs)
    if mode == "cc":
        out = _assemble_cc(res.results)
    elif mode in ("v2", "v3"):
        out = _assemble_v2(res.results)
    else:
        out = _assemble(res.results)
    return out, res


def kernel(boundary_tokens, W_b2b, b_b2b):
    out, _ = run(boundary_tokens, W_b2b, b_b2b)
    return out



# revision 26
# speedup vs baseline: 1.0733x; 1.0068x over previous
"""Trainium2 Bass kernel for nn_BulkSpaceGenerator.

Math: the fast-marching scan g_k = g_{k-1} + (1/(k+1))(c_k - g_{k-1}) starting
from c_0 yields the running mean g_k = mean(c_0..c_k); the mean over k of those
is sum_j w_j c_j with w_j = (1/K)(H_K - H_j) (harmonic numbers). Since
c_j = tokens @ W[:, j*D:(j+1)*D] + b[j*D:(j+1)*D], the whole module is

    out = tokens @ W_eff + b_eff,   W_eff = sum_j w_j W_j,  b_eff = sum_j w_j b_j

The kernel folds W -> W_eff on-device (DVE) and runs the (8192x1024)@(1024x1024)
matmul on the PE array, sharded over 8 cores as 4 feature-shards x 2
token-shards (minimizes per-core HBM traffic: W_slice + tokens/2 + out/8).

Layout per core (f in 0..3, t in 0..1, core = f*2 + t):
  tokT : (1024, 4096) f32  -- tokens^T slice, columns t*4096:(t+1)*4096
  wsl  : (1024, 2560) f32  -- W[:, j*1024 + f*256 : j*1024 + (f+1)*256], j-major
  bsl  : (256, 10)    f32  -- b[j*1024 + f*256 + d] transposed to (d, j)
  outT : (256, 4096)  f32  -- out^T slice (host reassembles full (4,2048,1024))
"""

import os
from contextlib import ExitStack

import numpy as np

import concourse.bass as bass
import concourse.tile as tile
from concourse import bacc, mybir
from concourse.bass_utils import run_bass_kernel_spmd

D_MODEL = 1024
BULK_DIM = 10
B, N = 4, 2048
BN = B * N                     # 8192 tokens
NCORES = 8
F_SHARDS = 4                   # feature shards (d dimension)
T_SHARDS = 2                   # token shards
DS = D_MODEL // F_SHARDS       # 256 output features per core
MS = BN // T_SHARDS            # 4096 tokens per core
KT = D_MODEL // 128            # 8 contraction k-tiles
DT = DS // 128                 # 2 output d-tiles of 128 per core
MCHUNK = 512                   # moving free dim per matmul
NMI = MS // MCHUNK             # 8 m-chunks per core

# w_j = (1/K) * (H_K - H_j), H_j = sum_{i=1..j} 1/i
_H = np.cumsum(1.0 / np.arange(1, BULK_DIM + 1))
W_COEF = ((_H[-1] - np.concatenate([[0.0], _H[:-1]])) / BULK_DIM).tolist()

# mode: "f32r" | "f32" | "bf16" keep f32 inputs on the wire; "f16" ships
# tokens and W as fp16 (half the load bytes, ~3.6e-4 rel err vs 1.5e-4 f32r)
# "cc": 8-way token shard + cooperative W_eff fold shared via AllGather
#       (dead: collectives have ~85us fixed cost under axon NRT)
# "v2": f16 wire + f16 out, kt-outer streaming in two m-phases
MODE = os.environ.get("BULK_KERNEL_MODE", "v2")

_BUILD_CACHE = {}

# ---------------- v2: f16 wire, kt-outer streaming, f16 out ----------------
# Same 4 feature-shards x 2 token-shards as f16 mode, but:
#  - DMA order per kt: W rows-chunk then phase-A token chunk, so the PE
#    starts accumulating at ~3us and chases the incoming stream
#  - W_eff fold runs in f16 (2x DVE rate); accuracy cost ~1e-4, fine
#  - psum is 8 banks of (128, 512): phase A = m 0..2047 kt-outer,
#    phase B = m 2048..4095 kt-outer once A's banks evict
#  - output ships f16 (halves store traffic; host upcasts)


def _build_v2() -> bass.Bass:
    f16 = mybir.dt.float16
    f32 = mybir.dt.float32

    nc = bacc.Bacc("TRN2", target_bir_lowering=False, debug=False,
                   num_devices=NCORES)
    tokT = nc.dram_tensor("tokT", [D_MODEL, MS], f16,
                          kind="ExternalInput").ap()
    wsl = nc.dram_tensor("wsl", [D_MODEL, BULK_DIM * DS], f16,
                         kind="ExternalInput").ap()
    bsl = nc.dram_tensor("bsl", [DS, BULK_DIM], f32, kind="ExternalInput").ap()
    outT = nc.dram_tensor("outT", [DS, MS], f16, kind="ExternalOutput").ap()

    HM = MS // 2                # 2048 tokens per phase
    NPH_MI = HM // MCHUNK       # 4 m-chunks per phase

    with tile.TileContext(nc) as tc, ExitStack() as ctx:
        sb = ctx.enter_context(tc.tile_pool(name="sb", bufs=1))
        out_pool = ctx.enter_context(tc.tile_pool(name="osb", bufs=4))
        psum_pool = ctx.enter_context(
            tc.tile_pool(name="psum", bufs=8, space="PSUM"))

        mult = mybir.AluOpType.mult
        add = mybir.AluOpType.add

        # PE warm operands
        zf = sb.tile([128, 512], f32)
        nc.vector.memset(zf[:], 0.0)
        zmm = sb.tile([128, 128], f16)
        nc.scalar.copy(zmm[:], zf[:, 0:128])
        zrhs = sb.tile([128, 512], f16)
        nc.scalar.copy(zrhs[:], zf[:])

        # ---- input stream (sync queue, program order == priority) ----
        wrs, toks = [], []
        for kt in range(KT):
            ksl = slice(kt * 128, (kt + 1) * 128)
            wr = sb.tile([128, BULK_DIM * DS], f16, name=f"wr{kt}")
            nc.sync.dma_start(wr[:], wsl[ksl, :])
            wrs.append(wr)
            tk = sb.tile([128, MS], f16, name=f"tk{kt}")
            nc.sync.dma_start(tk[:, 0:HM], tokT[ksl, 0:HM])
            toks.append(tk)
        for kt in range(KT):
            ksl = slice(kt * 128, (kt + 1) * 128)
            nc.sync.dma_start(toks[kt][:, HM:], tokT[ksl, HM:])

        # ---- fold W_eff per kt on DVE, all-f16 (chases the W DMAs) ----
        weffs = []
        for kt in range(KT):
            we = sb.tile([128, DS], f16, name=f"we{kt}")
            nc.vector.tensor_scalar_mul(we[:], wrs[kt][:, 0:DS], W_COEF[0])
            for j in range(1, BULK_DIM):
                nc.vector.scalar_tensor_tensor(
                    we[:], wrs[kt][:, j * DS:(j + 1) * DS], W_COEF[j],
                    we[:], mult, add)
            weffs.append(we)

        # ---- bias fold (f32, tiny) ----
        biases = []
        for dt_i in range(DT):
            bt = sb.tile([128, BULK_DIM], f32, name=f"bt{dt_i}")
            nc.scalar.dma_start(bt[:], bsl[dt_i * 128:(dt_i + 1) * 128, :])
            be = sb.tile([128, 1], f32, name=f"be{dt_i}")
            nc.vector.tensor_scalar_mul(be[:], bt[:, 0:1], W_COEF[0])
            for j in range(1, BULK_DIM):
                nc.vector.scalar_tensor_tensor(
                    be[:], bt[:, j:j + 1], W_COEF[j], be[:], mult, add)
            biases.append(be)

        def evict(ps, dt_i, msl):
            ot = out_pool.tile([128, MCHUNK], f16, name="ot", tag="ot")
            if dt_i == 0:
                nc.scalar.add(ot[:], ps[:], biases[dt_i][:])
            else:
                nc.vector.tensor_scalar_add(ot[:], ps[:], biases[dt_i][:, 0:1])
            nc.gpsimd.dma_start(outT[dt_i * 128:(dt_i + 1) * 128, msl], ot[:])

        # keep the PE HAM clock warm until the first weff lands
        ps_warm = psum_pool.tile([128, MCHUNK], f32, name="ps", tag="ps")
        for _ in range(16):
            nc.tensor.matmul(ps_warm[:], lhsT=zmm[:], rhs=zrhs[:],
                             start=False, stop=False)

        # ---- phase A: m 0..HM, kt-outer over 8 live psum groups ----
        psA = [ps_warm] + [
            psum_pool.tile([128, MCHUNK], f32, name="ps", tag="ps")
            for _ in range(NPH_MI * DT - 1)]
        for kt in range(KT):
            for dt_i in range(DT):
                for mi in range(NPH_MI):
                    nc.tensor.matmul(
                        psA[mi * DT + dt_i][:],
                        lhsT=weffs[kt][:, dt_i * 128:(dt_i + 1) * 128],
                        rhs=toks[kt][:, mi * MCHUNK:(mi + 1) * MCHUNK],
                        start=(kt == 0), stop=(kt == KT - 1))
        for mi in range(NPH_MI):
            for dt_i in range(DT):
                evict(psA[mi * DT + dt_i], dt_i,
                      slice(mi * MCHUNK, (mi + 1) * MCHUNK))

        # ---- phase B: m HM..MS ----
        # phase B: tokens are resident by now, so run single-group
        # accumulation chains and evict each group immediately -- spreads
        # the evict+store tail across phase B instead of piling it at the
        # end of the kernel.
        for mi in range(NPH_MI):
            for dt_i in range(DT):
                ps = psum_pool.tile([128, MCHUNK], f32, name="ps", tag="ps")
                m0 = HM + mi * MCHUNK
                for kt in range(KT):
                    nc.tensor.matmul(
                        ps[:],
                        lhsT=weffs[kt][:, dt_i * 128:(dt_i + 1) * 128],
                        rhs=toks[kt][:, m0:m0 + MCHUNK],
                        start=(kt == 0), stop=(kt == KT - 1))
                evict(ps, dt_i, slice(m0, m0 + MCHUNK))

    nc.compile()
    return nc


def _make_in_maps_v2(boundary_tokens, W_b2b, b_b2b, wsl3d=False):
    tok = np.ascontiguousarray(
        np.asarray(boundary_tokens, dtype=np.float32)
        .reshape(BN, D_MODEL).T.astype(np.float16))
    W = np.asarray(W_b2b, np.float32).astype(np.float16).reshape(
        D_MODEL, BULK_DIM, D_MODEL)
    b = np.asarray(b_b2b, np.float32).reshape(BULK_DIM, D_MODEL)
    wshape = ((D_MODEL, BULK_DIM, DS) if wsl3d
              else (D_MODEL, BULK_DIM * DS))
    in_maps = []
    for c in range(NCORES):
        f, t = divmod(c, T_SHARDS)
        dsl = slice(f * DS, (f + 1) * DS)
        in_maps.append({
            "tokT": np.ascontiguousarray(tok[:, t * MS:(t + 1) * MS]),
            "wsl": np.ascontiguousarray(
                W[:, :, dsl].reshape(*wshape)),
            "bsl": np.ascontiguousarray(b[:, dsl].T),
        })
    return in_maps


def _assemble_v2(results):
    out = np.empty((BN, D_MODEL), dtype=np.float32)
    for c in range(NCORES):
        f, t = divmod(c, T_SHARDS)
        out[t * MS:(t + 1) * MS, f * DS:(f + 1) * DS] = results[c]["outT"].T
    return out.reshape(B, N, D_MODEL)


# ------- v4: v2 + fold restructured to fit the DMA arrival cadence -------
# The W_eff fold is 2 ALU ops/elem; v2's 10-step STT chain costs 4.25us
# per kt-tile on the DVE vs the 3.26us DMA arrival cadence, so it paced
# the whole kernel. v4 folds each kt as:
#   vector : tmp[0:6] = W[0:6] * C  (one big tensor_mul vs a memset
#            coefficient tile), then an in-place pairwise add tree
#   scalar : tmp[6..9] = cj*Wj      (4 ACT scale-copies)
# ~2.8us/kt on vector, ~1.6us/kt on scalar (gpsimd can't run DVE ops on
# real TRN2 - ISA check rejects Pool-engine TensorScalar).


def _build_v3() -> bass.Bass:
    f16 = mybir.dt.float16
    f32 = mybir.dt.float32

    nc = bacc.Bacc("TRN2", target_bir_lowering=False, debug=False,
                   num_devices=NCORES)
    tokT = nc.dram_tensor("tokT", [D_MODEL, MS], f16,
                          kind="ExternalInput").ap()
    wsl = nc.dram_tensor("wsl", [D_MODEL, BULK_DIM, DS], f16,
                         kind="ExternalInput").ap()
    bsl = nc.dram_tensor("bsl", [DS, BULK_DIM], f32, kind="ExternalInput").ap()
    outT = nc.dram_tensor("outT", [DS, MS], f16, kind="ExternalOutput").ap()

    HM = MS // 2
    NPH_MI = HM // MCHUNK

    with tile.TileContext(nc) as tc, ExitStack() as ctx:
        sb = ctx.enter_context(tc.tile_pool(name="sb", bufs=1))
        tmp_pool = ctx.enter_context(tc.tile_pool(name="tmp", bufs=2))
        out_pool = ctx.enter_context(tc.tile_pool(name="osb", bufs=4))
        psum_pool = ctx.enter_context(
            tc.tile_pool(name="psum", bufs=8, space="PSUM"))

        mult = mybir.AluOpType.mult
        add = mybir.AluOpType.add

        zf = sb.tile([128, 512], f32)
        nc.vector.memset(zf[:], 0.0)
        zmm = sb.tile([128, 128], f16)
        nc.scalar.copy(zmm[:], zf[:, 0:128])
        zrhs = sb.tile([128, 512], f16)
        nc.scalar.copy(zrhs[:], zf[:])

        # coefficient tile for the one-op fold multiply (j = 0..5)
        cco = sb.tile([128, 6, DS], f16)
        for j in range(6):
            nc.vector.memset(cco[:, j, :], float(W_COEF[j]))

        # W chunks run one step ahead of the phase-A token chunks so the
        # fold is never the late dependency of a kt batch.
        wrs = [sb.tile([128, BULK_DIM, DS], f16, name=f"wr{kt}")
               for kt in range(KT)]
        toks = [sb.tile([128, MS], f16, name=f"tk{kt}") for kt in range(KT)]

        def ksl(kt):
            return slice(kt * 128, (kt + 1) * 128)

        nc.sync.dma_start(wrs[0][:], wsl[ksl(0), :, :])
        for kt in range(1, KT):
            nc.sync.dma_start(wrs[kt][:], wsl[ksl(kt), :, :])
            nc.sync.dma_start(toks[kt - 1][:, 0:HM], tokT[ksl(kt - 1), 0:HM])
        nc.sync.dma_start(toks[KT - 1][:, 0:HM], tokT[ksl(KT - 1), 0:HM])

        for kt in range(KT):
            nc.sync.dma_start(toks[kt][:, HM:], tokT[ksl(kt), HM:])

        # ---- fold: big-op multiply + pairwise add tree ----
        weffs = []
        for kt in range(KT):
            wr = wrs[kt]
            tmp = tmp_pool.tile([128, BULK_DIM, DS], f16, name="tmp",
                                tag="tmp")
            nc.vector.tensor_mul(tmp[:, 0:6, :], wr[:, 0:6, :], cco[:])
            for j in range(6, BULK_DIM):
                nc.scalar.mul(tmp[:, j, :], wr[:, j, :], W_COEF[j])
            nc.vector.tensor_add(tmp[:, 0:5, :], tmp[:, 0:5, :],
                                 tmp[:, 5:10, :])
            nc.vector.tensor_add(tmp[:, 0:2, :], tmp[:, 0:2, :],
                                 tmp[:, 2:4, :])
            nc.vector.tensor_add(tmp[:, 0, :], tmp[:, 0, :], tmp[:, 1, :])
            we = sb.tile([128, DS], f16, name=f"we{kt}")
            nc.vector.tensor_add(we[:], tmp[:, 0, :], tmp[:, 4, :])
            weffs.append(we)

        biases = []
        for dt_i in range(DT):
            bt = sb.tile([128, BULK_DIM], f32, name=f"bt{dt_i}")
            nc.scalar.dma_start(bt[:], bsl[dt_i * 128:(dt_i + 1) * 128, :])
            be = sb.tile([128, 1], f32, name=f"be{dt_i}")
            nc.vector.tensor_scalar_mul(be[:], bt[:, 0:1], W_COEF[0])
            for j in range(1, BULK_DIM):
                nc.vector.scalar_tensor_tensor(
                    be[:], bt[:, j:j + 1], W_COEF[j], be[:], mult, add)
            biases.append(be)

        def evict(ps, dt_i, msl):
            ot = out_pool.tile([128, MCHUNK], f16, name="ot", tag="ot")
            if dt_i == 0:
                nc.scalar.add(ot[:], ps[:], biases[dt_i][:])
            else:
                nc.vector.tensor_scalar_add(ot[:], ps[:], biases[dt_i][:, 0:1])
            nc.sync.dma_start(outT[dt_i * 128:(dt_i + 1) * 128, msl], ot[:])

        ps_warm = psum_pool.tile([128, MCHUNK], f32, name="ps", tag="ps")
        for _ in range(40):
            nc.tensor.matmul(ps_warm[:], lhsT=zmm[:], rhs=zrhs[:],
                             start=False, stop=False)

        psA = [ps_warm] + [
            psum_pool.tile([128, MCHUNK], f32, name="ps", tag="ps")
            for _ in range(NPH_MI * DT - 1)]
        for kt in range(KT):
            for dt_i in range(DT):
                for mi in range(NPH_MI):
                    nc.tensor.matmul(
                        psA[mi * DT + dt_i][:],
                        lhsT=weffs[kt][:, dt_i * 128:(dt_i + 1) * 128],
                        rhs=toks[kt][:, mi * MCHUNK:(mi + 1) * MCHUNK],
                        start=(kt == 0), stop=(kt == KT - 1))
        for mi in range(NPH_MI):
            for dt_i in range(DT):
                evict(psA[mi * DT + dt_i], dt_i,
                      slice(mi * MCHUNK, (mi + 1) * MCHUNK))

        # phase B: tokens are resident by now, so run single-group
        # accumulation chains and evict each group immediately -- spreads
        # the evict+store tail across phase B instead of piling it at the
        # end of the kernel.
        for mi in range(NPH_MI):
            for dt_i in range(DT):
                ps = psum_pool.tile([128, MCHUNK], f32, name="ps", tag="ps")
                m0 = HM + mi * MCHUNK
                for kt in range(KT):
                    nc.tensor.matmul(
                        ps[:],
                        lhsT=weffs[kt][:, dt_i * 128:(dt_i + 1) * 128],
                        rhs=toks[kt][:, m0:m0 + MCHUNK],
                        start=(kt == 0), stop=(kt == KT - 1))
                evict(ps, dt_i, slice(m0, m0 + MCHUNK))

    nc.compile()
    return nc


# ---------------- cc mode: cooperative fold + AllGather ----------------
# Each core folds 128 rows of W_eff from its 2.5MB W row-slice, the 8
# partial (128, 1024) results are AllGathered into the full (1024, 1024)
# W_eff, and each core then multiplies its own 1024-token shard against
# it. Per-core HBM traffic: 2.5 (W) + 2 (tok) + 0.25 + 2 (cc) + 2 (out)
# ~= 8.75MB vs 17.8MB for the f16 shard-by-feature layout.
CC_MS = BN // NCORES            # 1024 tokens per core
CC_MCHUNK = 512                 # moving free dim per matmul
CC_NMI = CC_MS // CC_MCHUNK     # 2 m-waves
CC_DT = D_MODEL // 128          # 8 output d2 tiles (full feature dim)
CC_PREWARM = 60                 # PE warm dummies while fold+gather runs


def _build_cc() -> bass.Bass:
    f16 = mybir.dt.float16
    f32 = mybir.dt.float32

    nc = bacc.Bacc("TRN2", target_bir_lowering=False, debug=False,
                   num_devices=NCORES)
    tokT = nc.dram_tensor("tokT", [128, KT * CC_MS], f16,
                          kind="ExternalInput").ap()
    wsl = nc.dram_tensor("wsl", [128, BULK_DIM * D_MODEL], f16,
                         kind="ExternalInput").ap()
    bsl = nc.dram_tensor("bsl", [128, BULK_DIM * CC_DT], f32,
                         kind="ExternalInput").ap()
    outT = nc.dram_tensor("outT", [D_MODEL, CC_MS], f16,
                          kind="ExternalOutput").ap()

    with tile.TileContext(nc) as tc, ExitStack() as ctx:
        sb = ctx.enter_context(tc.tile_pool(name="sb", bufs=1))
        out_pool = ctx.enter_context(tc.tile_pool(name="osb", bufs=4))
        psum_pool = ctx.enter_context(
            tc.tile_pool(name="psum", bufs=8, space="PSUM"))
        dram = ctx.enter_context(tc.tile_pool(name="dram", bufs=1,
                                              space="DRAM"))

        mult = mybir.AluOpType.mult
        add = mybir.AluOpType.add

        # PE-warm operands (zeros). memset f32 then rounding-copy to f16.
        zf = sb.tile([128, 512], f32)
        nc.vector.memset(zf[:], 0.0)
        zmm = sb.tile([128, 128], f16)
        nc.scalar.copy(zmm[:], zf[:, 0:128])
        zrhs = sb.tile([128, 512], f16)
        nc.scalar.copy(zrhs[:], zf[:])

        # ---- load W row-slice (sync queue, ahead of tokens) ----
        wr = sb.tile([128, BULK_DIM * D_MODEL], f16)
        NCH = 5  # 2 k-groups per chunk
        for ch in range(NCH):
            csl = slice(ch * 2 * D_MODEL, (ch + 1) * 2 * D_MODEL)
            nc.sync.dma_start(wr[:, csl], wsl[:, csl])

        # tokens: 2 chunks of 4 k-tiles each, behind W on the same queue
        tok = sb.tile([128, KT * CC_MS], f16)
        half = KT * CC_MS // 2
        nc.sync.dma_start(tok[:, 0:half], tokT[:, 0:half])
        nc.sync.dma_start(tok[:, half:], tokT[:, half:])

        # ---- fold W_eff rows on DVE (chases the W DMA chunks) ----
        we = sb.tile([128, D_MODEL], f32)
        nc.vector.tensor_scalar_mul(we[:], wr[:, 0:D_MODEL], W_COEF[0])
        for j in range(1, BULK_DIM - 1):
            nc.vector.scalar_tensor_tensor(
                we[:], wr[:, j * D_MODEL:(j + 1) * D_MODEL], W_COEF[j],
                we[:], mult, add)
        wc = sb.tile([128, D_MODEL], f16)
        j = BULK_DIM - 1
        nc.vector.scalar_tensor_tensor(
            wc[:], wr[:, j * D_MODEL:(j + 1) * D_MODEL], W_COEF[j],
            we[:], mult, add)

        # ---- share the fold: bounce to DRAM, AllGather over 8 cores ----
        cc_in = dram.tile([128, D_MODEL], f16)
        cc_out = dram.tile([NCORES * 128, D_MODEL], f16, addr_space="Shared")
        nc.gpsimd.dma_start(cc_in[:], wc[:])
        nc.gpsimd.collective_compute(
            "AllGather", mybir.AluOpType.bypass,
            replica_groups=[list(range(NCORES))],
            ins=[cc_in.opt()], outs=[cc_out.opt()])

        # read the full W_eff back (scalar queue; fires as CC completes)
        weff = sb.tile([128, KT * D_MODEL], f16)
        for kt in range(KT):
            nc.scalar.dma_start(
                weff[:, kt * D_MODEL:(kt + 1) * D_MODEL],
                cc_out[kt * 128:(kt + 1) * 128, :])

        # ---- fold bias: be[p, j] = sum_k w_k bsl[p, k*8+j] ----
        bt = sb.tile([128, BULK_DIM * CC_DT], f32)
        nc.sync.dma_start(bt[:], bsl[:])
        be = sb.tile([128, CC_DT], f32)
        nc.vector.tensor_scalar_mul(be[:], bt[:, 0:CC_DT], W_COEF[0])
        for j in range(1, BULK_DIM):
            nc.vector.scalar_tensor_tensor(
                be[:], bt[:, j * CC_DT:(j + 1) * CC_DT], W_COEF[j], be[:],
                mult, add)

        # ---- matmuls: kt-outer per m-wave, chasing the gather ----
        ps0 = [psum_pool.tile([128, CC_MCHUNK], f32, name="ps", tag="ps")
               for _ in range(CC_DT)]
        for _ in range(CC_PREWARM):
            nc.tensor.matmul(ps0[0][:], lhsT=zmm[:], rhs=zrhs[:],
                             start=False, stop=False)

        def evict(ps, d2t, mi):
            ot = out_pool.tile([128, CC_MCHUNK], f16, name="ot", tag="ot")
            msl = slice(mi * CC_MCHUNK, (mi + 1) * CC_MCHUNK)
            if d2t % 2 == 0:
                nc.scalar.add(ot[:], ps[:], be[:, d2t:d2t + 1])
            else:
                nc.vector.tensor_scalar_add(ot[:], ps[:], be[:, d2t:d2t + 1])
            nc.gpsimd.dma_start(outT[d2t * 128:(d2t + 1) * 128, msl], ot[:])

        for mi in range(CC_NMI):
            psw = ps0 if mi == 0 else [
                psum_pool.tile([128, CC_MCHUNK], f32, name="ps", tag="ps")
                for _ in range(CC_DT)]
            msl = slice(mi * CC_MCHUNK, (mi + 1) * CC_MCHUNK)
            for kt in range(KT):
                for d2t in range(CC_DT):
                    nc.tensor.matmul(
                        psw[d2t][:],
                        lhsT=weff[:, kt * D_MODEL + d2t * 128:
                                  kt * D_MODEL + (d2t + 1) * 128],
                        rhs=tok[:, kt * CC_MS + mi * CC_MCHUNK:
                                kt * CC_MS + (mi + 1) * CC_MCHUNK],
                        start=(kt == 0), stop=(kt == KT - 1))
            for d2t in range(CC_DT):
                evict(psw[d2t], d2t, mi)

    nc.compile()
    return nc


def _make_in_maps_cc(boundary_tokens, W_b2b, b_b2b):
    tok16 = np.asarray(boundary_tokens, np.float32).reshape(
        BN, D_MODEL).astype(np.float16)
    # tok_wide[c][p, kt*1024 + m] = tok16[c*1024 + m, kt*128 + p]
    tw = tok16.reshape(NCORES, CC_MS, KT, 128).transpose(0, 3, 2, 1)
    W16 = np.asarray(W_b2b, np.float32).astype(np.float16)
    b = np.asarray(b_b2b, np.float32).reshape(BULK_DIM, CC_DT, 128)
    bsl = np.ascontiguousarray(
        b.transpose(2, 0, 1).reshape(128, BULK_DIM * CC_DT))
    in_maps = []
    for c in range(NCORES):
        in_maps.append({
            "tokT": np.ascontiguousarray(tw[c].reshape(128, KT * CC_MS)),
            "wsl": np.ascontiguousarray(W16[c * 128:(c + 1) * 128, :]),
            "bsl": bsl,
        })
    return in_maps


def _assemble_cc(results):
    out = np.empty((BN, D_MODEL), dtype=np.float32)
    for c in range(NCORES):
        out[c * CC_MS:(c + 1) * CC_MS, :] = results[c]["outT"].T
    return out.reshape(B, N, D_MODEL)


def _build(mode: str) -> bass.Bass:
    f32 = mybir.dt.float32
    bf16 = mybir.dt.bfloat16
    wire_dt = mybir.dt.float16 if mode in ("f16",) else f32

    nc = bacc.Bacc("TRN2", target_bir_lowering=False, debug=False,
                   num_devices=NCORES)
    tokT = nc.dram_tensor("tokT", [D_MODEL, MS], wire_dt,
                          kind="ExternalInput").ap()
    wsl = nc.dram_tensor("wsl", [D_MODEL, BULK_DIM * DS], wire_dt,
                         kind="ExternalInput").ap()
    bsl = nc.dram_tensor("bsl", [DS, BULK_DIM], f32, kind="ExternalInput").ap()
    outT = nc.dram_tensor("outT", [DS, MS], f32, kind="ExternalOutput").ap()

    with tile.TileContext(nc) as tc, ExitStack() as ctx:
        wraw_pool = ctx.enter_context(
            tc.tile_pool(name="wraw",
                         bufs=KT if mode in ("f16",) else 2))
        weff_pool = ctx.enter_context(tc.tile_pool(name="weff", bufs=KT))
        tok_pool = ctx.enter_context(tc.tile_pool(name="tok", bufs=KT))
        bias_pool = ctx.enter_context(tc.tile_pool(name="bias", bufs=2 * DT))
        psum_pool = ctx.enter_context(
            tc.tile_pool(name="psum", bufs=8, space="PSUM"))
        out_pool = ctx.enter_context(tc.tile_pool(name="osb", bufs=4))
        weffc_pool = None
        if mode != "f32":
            weffc_pool = ctx.enter_context(tc.tile_pool(name="weffc", bufs=KT))

        mult = mybir.AluOpType.mult
        add = mybir.AluOpType.add
        mm_dtype = {"bf16": bf16, "f32r": mybir.dt.float32r, "f32": f32,
                    "f16": mybir.dt.float16}[mode]

        # zero operands for PE-warming no-op matmuls (memset can't write f32r
        # directly; produce via a rounding copy). zrhs is independent of any
        # input DMA so warm-up can start immediately.
        zf = bias_pool.tile([128, 512], f32, tag="zf")
        nc.vector.memset(zf[:], 0.0)
        # casts on ACT (idle early) so they don't delay the DVE fold chains
        # (ACT Copy with an f32r out dtype is unverified -> DVE for f32r)
        zcast = nc.scalar if mode == "f16" else nc.vector
        zmm = bias_pool.tile([128, 128], mm_dtype, tag="zmm")
        zcast.copy(zmm[:], zf[:, 0:128]) if mode == "f16" else \
            nc.vector.tensor_copy(zmm[:], zf[:, 0:128])
        zrhs = bias_pool.tile([128, 512], mm_dtype, tag="zrhs")
        zcast.copy(zrhs[:], zf[:]) if mode == "f16" else \
            nc.vector.tensor_copy(zrhs[:], zf[:])

        # ---- per k-tile: load W slice, fold W_eff, load tokens ----
        toks = []
        weffs = []
        for kt in range(KT):
            ksl = slice(kt * 128, (kt + 1) * 128)
            wr = wraw_pool.tile([128, BULK_DIM * DS], wire_dt)
            if mode in ("f16",):
                # split columns so the fold chain (j ascending) starts as
                # soon as the first half lands (subtile deps)
                hw = BULK_DIM * DS // 2
                nc.scalar.dma_start(wr[:, 0:hw], wsl[ksl, 0:hw])
                nc.scalar.dma_start(wr[:, hw:], wsl[ksl, hw:])
            else:
                nc.gpsimd.dma_start(wr[:], wsl[ksl, :])

            tk = tok_pool.tile([128, MS], mm_dtype)
            if mode in ("f16",):
                # no cast needed -> HWDGE queue, decoupled from W-load waits
                nc.sync.dma_start(tk[:], tokT[ksl, :])
            else:
                nc.gpsimd.dma_start(tk[:], tokT[ksl, :])  # SWDGE rounding cast
            toks.append(tk)

            fold = nc.vector
            we = weff_pool.tile([128, DS], f32)
            fold.tensor_scalar_mul(we[:], wr[:, 0:DS], W_COEF[0])
            for j in range(1, BULK_DIM - 1):
                fold.scalar_tensor_tensor(
                    we[:], wr[:, j * DS:(j + 1) * DS], W_COEF[j], we[:],
                    mult, add)
            j = BULK_DIM - 1
            if mode == "f32":
                fold.scalar_tensor_tensor(
                    we[:], wr[:, j * DS:(j + 1) * DS], W_COEF[j], we[:],
                    mult, add)
                weffs.append(we)
            else:
                # final fold step writes the matmul dtype directly (rounds)
                wc = weffc_pool.tile([128, DS], mm_dtype)
                fold.scalar_tensor_tensor(
                    wc[:], wr[:, j * DS:(j + 1) * DS], W_COEF[j], we[:],
                    mult, add)
                weffs.append(wc)

        # ---- fold bias: beff[d] = sum_j w_j b[j*D + d], per-partition ----
        biases = []
        bfold = nc.vector
        for dt_i in range(DT):
            bt = bias_pool.tile([128, BULK_DIM], f32, tag="bt")
            nc.sync.dma_start(bt[:], bsl[dt_i * 128:(dt_i + 1) * 128, :])
            be = bias_pool.tile([128, 1], f32, tag="be")
            bfold.tensor_scalar_mul(be[:], bt[:, 0:1], W_COEF[0])
            for j in range(1, BULK_DIM):
                bfold.scalar_tensor_tensor(
                    be[:], bt[:, j:j + 1], W_COEF[j], be[:], mult, add)
            biases.append(be)

        # ---- matmul: kt-outer in 2 halves (8 live psum banks each) ----
        # Between k-steps of the first half the PE is supply-gated on DMA;
        # zero-weight no-op matmuls keep its HAM clock at 8/8 (idle >3.4us
        # re-throttles the PE to 1.2 GHz).
        n_dummy = 4 if mode != "f32" else 0
        n_prewarm = 40 if mode != "f32" else 0
        half_mi = NMI // 2

        def evict(ps, dt_i, msl):
            ot = out_pool.tile([128, MCHUNK], f32, name="ot", tag="ot")
            if dt_i == 0:
                nc.scalar.add(ot[:], ps[:], biases[dt_i][:])
                nc.scalar.dma_start(
                    outT[dt_i * 128:(dt_i + 1) * 128, msl], ot[:])
            else:
                nc.vector.tensor_scalar_add(
                    ot[:], ps[:], biases[dt_i][:, 0:1])
                nc.sync.dma_start(
                    outT[dt_i * 128:(dt_i + 1) * 128, msl], ot[:])

        # half 0: kt-outer (matmuls chase the incoming DMA stream)
        psums = [[psum_pool.tile([128, MCHUNK], f32, name="ps", tag="ps")
                  for _ in range(DT)] for _ in range(half_mi)]
        # warm the PE's HAM clock before the first real matmul; these touch
        # only memset tiles, so they run during the load phase
        for _ in range(n_prewarm):
            nc.tensor.matmul(psums[0][0][:], lhsT=zmm[:], rhs=zrhs[:],
                             start=False, stop=False)
        for kt in range(KT):
            for mi_l in range(half_mi):
                msl = slice(mi_l * MCHUNK, (mi_l + 1) * MCHUNK)
                for dt_i in range(DT):
                    nc.tensor.matmul(
                        psums[mi_l][dt_i][:],
                        lhsT=weffs[kt][:, dt_i * 128:(dt_i + 1) * 128],
                        rhs=toks[kt][:, msl],
                        start=(kt == 0), stop=(kt == KT - 1))
            if kt < KT - 1:
                for _ in range(n_dummy):
                    nc.tensor.matmul(psums[0][0][:], lhsT=zmm[:], rhs=zrhs[:],
                                     start=False, stop=False)
        for mi_l in range(half_mi):
            msl = slice(mi_l * MCHUNK, (mi_l + 1) * MCHUNK)
            for dt_i in range(DT):
                evict(psums[mi_l][dt_i], dt_i, msl)

        # half 1: everything is SBUF-resident by now, so go psum-outer --
        # each output group evicts right after its 8 matmuls, overlapping
        # the remaining groups' matmuls instead of trailing them all
        for mi_l in range(half_mi):
            mi = half_mi + mi_l
            msl = slice(mi * MCHUNK, (mi + 1) * MCHUNK)
            for dt_i in range(DT):
                ps = psum_pool.tile([128, MCHUNK], f32, name="ps", tag="ps")
                for kt in range(KT):
                    nc.tensor.matmul(
                        ps[:],
                        lhsT=weffs[kt][:, dt_i * 128:(dt_i + 1) * 128],
                        rhs=toks[kt][:, msl],
                        start=(kt == 0), stop=(kt == KT - 1))
                evict(ps, dt_i, msl)

    nc.compile()
    return nc


_BUILDERS = {"cc": _build_cc, "v2": _build_v2, "v3": _build_v3}


def _get_nc(mode: str) -> bass.Bass:
    if mode not in _BUILD_CACHE:
        _BUILD_CACHE[mode] = _BUILDERS.get(mode, lambda: _build(mode))()
    return _BUILD_CACHE[mode]


def _make_in_maps(boundary_tokens, W_b2b, b_b2b, mode):
    wire = np.float16 if mode == "f16" else np.float32
    tok = np.ascontiguousarray(
        np.asarray(boundary_tokens, dtype=np.float32)
        .reshape(BN, D_MODEL).T.astype(wire))
    W = np.asarray(W_b2b, dtype=np.float32).astype(wire).reshape(
        D_MODEL, BULK_DIM, D_MODEL)
    b = np.asarray(b_b2b, dtype=np.float32).reshape(BULK_DIM, D_MODEL)
    in_maps = []
    for c in range(NCORES):
        f, t = divmod(c, T_SHARDS)
        dsl = slice(f * DS, (f + 1) * DS)
        in_maps.append({
            "tokT": np.ascontiguousarray(tok[:, t * MS:(t + 1) * MS]),
            "wsl": np.ascontiguousarray(
                W[:, :, dsl].reshape(D_MODEL, BULK_DIM * DS)),
            "bsl": np.ascontiguousarray(b[:, dsl].T),
        })
    return in_maps


def _assemble(results):
    out = np.empty((BN, D_MODEL), dtype=np.float32)
    for c in range(NCORES):
        f, t = divmod(c, T_SHARDS)
        out[t * MS:(t + 1) * MS, f * DS:(f + 1) * DS] = results[c]["outT"].T
    return out.reshape(B, N, D_MODEL)


def run(boundary_tokens, W_b2b, b_b2b, mode=None, **spmd_kwargs):
    mode = mode or MODE
    nc = _get_nc(mode)
    if mode == "cc":
        in_maps = _make_in_maps_cc(boundary_tokens, W_b2b, b_b2b)
    elif mode in ("v2", "v3"):
        in_maps = _make_in_maps_v2(boundary_tokens, W_b2b, b_b2b,
                                   wsl3d=(mode == "v3"))
    else:
        in_maps = _make_in_maps(boundary_tokens, W_b2b, b_b2b, mode)
    res = run_bass_kernel_spmd(nc, in_maps, list(range(NCORES)), **spmd_kwargs)
    if mode == "cc":
        out = _assemble_cc(res.results)
    elif mode in ("v2", "v3"):
        out = _assemble_v2(res.results)
    else:
        out = _assemble(res.results)
    return out, res


def kernel(boundary_tokens, W_b2b, b_b2b):
    out, _ = run(boundary_tokens, W_b2b, b_b2b)
    return out

